# revision 9
# baseline (speedup 1.0000x reference)
"""Trainium2 Bass kernel for nn_AttGNN (3-layer GCN+attention network).

Sharding: 8 NeuronCores = 4 batch elements x 2-way node split (each core
owns 1024 of the 2048 nodes). Pair AllGathers supply the full node set
where needed (GCN aggregation input, attention K/V).

GCN message passing is reformulated as a dense matmul against the
host-precomputed symmetric-normalized adjacency (self-loop folded into
the diagonal), reordered as (M @ h) @ Wg so only the natural-layout h is
needed. The GroupNorm per-channel gain is folded into Wg on the host.
"""

import sys

sys.path.insert(0, "/opt/trn_rl_repo")

import numpy as np
import ml_dtypes

B, V, E = 4, 2048, 65536
F_IN, HID, COND, ENC, HEADS, NL = 256, 256, 64, 128, 8, 3
FF = 4 * HID
VH = V // 2          # nodes owned per core
DH = HID // HEADS    # 32
F0 = F_IN + COND     # 320, first-layer input width
NCORES = 8
KVFLAT = HID * VH + VH * HID   # flattened kT + v_nat halves, in elements
ST = V // 128        # 16 s-tiles over full V
VS = VH // 128       # 8 v-slices over own nodes
QC = VH // 512       # 2 q-chunks of 512

_bf = ml_dtypes.bfloat16


def _np(a):
    return np.asarray(a)


def _host_prep(x, cond_x, edge_index, t, params):
    """Build the per-core input maps (all numpy)."""
    x = _np(x).astype(np.float32)
    cond_x = _np(cond_x).astype(np.float32)
    ei = _np(edge_index).astype(np.int64)
    t = _np(t).astype(np.float32)

    src, dst = ei[0], ei[1]
    deg = np.zeros(V, np.float32)
    np.add.at(deg, dst, np.float32(1.0))
    deg += 1.0
    dinv = (1.0 / np.sqrt(deg)).astype(np.float32)

    # MT[s, d] = A[d, s]; A = normalized adjacency + diag(1/deg)
    MT = np.zeros((V, V), np.float32)
    np.add.at(MT, (src, dst), dinv[src] * dinv[dst])
    MT[np.arange(V), np.arange(V)] += 1.0 / deg
    msum = MT.sum(axis=0)  # row sums of A (for optional gn_b term)

    gn_g = _np(params["gn_g"]).astype(np.float32)
    gn_b = _np(params["gn_b"]).astype(np.float32)
    Wfilm = _np(params["Wfilm"]).astype(np.float32)
    bfilm = _np(params["bfilm"]).astype(np.float32)

    film = t @ Wfilm + bfilm          # [B, 2*HID]
    gm, bt = film[:, :HID], film[:, HID:]

    L = [
        {k: _np(v).astype(np.float32) for k, v in params["layers"][i].items()}
        for i in range(NL)
    ]

    # fold gn_g into Wg of layer 1 (the only gnorm'ed gcn input)
    Wg_eff = [L[0]["Wg"], gn_g[:, None] * L[1]["Wg"], L[2]["Wg"]]
    w0 = gn_b @ L[1]["Wg"]            # [HID]; nonzero only if gn_b != 0
    has_gnb = bool(np.any(gn_b != 0.0))

    condT = np.stack(
        [(cond_x @ L[i]["We"] + L[i]["be"]).T.astype(np.float32) for i in range(NL)]
    )  # [NL, HID, V]

    rows = []
    colb_idx = {}

    def add_col(name, vec):
        vec = vec.reshape(-1, 128)
        colb_idx[name] = len(rows)
        rows.extend(list(vec))

    for i in range(NL):
        add_col(f"bg{i}", L[i]["bg"])
        add_col(f"bq{i}", L[i]["bq"])
        add_col(f"bk{i}", L[i]["bk"])
        add_col(f"bo{i}", L[i]["bo"])
        add_col(f"bf2_{i}", L[i]["bf2"])
        add_col(f"lng{i}", L[i]["ln_g"])
        add_col(f"lnb{i}", L[i]["ln_b"])
        add_col(f"bf1_{i}", L[i]["bf1"])
    add_col("gm0", gm[0])   # per-core batch values patched in kernel()
    add_col("bt0", bt[0])
    colb = np.stack(rows).astype(np.float32)

    natb = np.stack(
        [np.broadcast_to(L[i]["bv"], (128, HID)) for i in range(NL)]
        + [np.broadcast_to(L[i]["bl"], (128, HID)) for i in range(NL)]
    ).astype(np.float32)

    wg0 = Wg_eff[0].astype(_bf)
    wg12 = np.stack([Wg_eff[1], Wg_eff[2]]).astype(_bf)
    wq = np.stack([L[i]["Wq"] for i in range(NL)]).astype(_bf)
    wk = np.stack([L[i]["Wk"] for i in range(NL)]).astype(_bf)
    wv = np.stack([L[i]["Wv"] for i in range(NL)]).astype(_bf)
    wo = np.stack([L[i]["Wo"] for i in range(NL)]).astype(_bf)
    wl = np.stack([L[i]["Wl"] for i in range(NL)]).astype(_bf)
    wf1 = np.stack([L[i]["Wf1"] for i in range(NL)]).astype(_bf)
    wf2 = np.stack([L[i]["Wf2"] for i in range(NL)]).astype(_bf)

    shared = {
        "colb_idx": colb_idx,
        "has_gnb": has_gnb,
        "gm": gm,
        "bt": bt,
        "ncolb": colb.shape[0],
    }

    in_maps = []
    for core in range(NCORES):
        b, r = core // 2, core % 2
        lo, hi = r * VH, (r + 1) * VH
        cb = colb.copy()
        cb[colb_idx["gm0"] : colb_idx["gm0"] + 2] = gm[b].reshape(2, 128)
        cb[colb_idx["bt0"] : colb_idx["bt0"] + 2] = bt[b].reshape(2, 128)
        m = {
            "h0": np.concatenate([x[b], cond_x], axis=-1).astype(_bf),
            "mt": MT[:, lo:hi].astype(_bf),
            "xskip": x[b, lo:hi].astype(np.float32),
            "cond": condT[:, :, lo:hi].astype(np.float32),
            "wg0": wg0,
            "wg12": wg12,
            "wq": wq,
            "wk": wk,
            "wv": wv,
            "wo": wo,
            "wl": wl,
            "wf1": wf1,
            "wf2": wf2,
            "colb": cb,
            "natb": natb,
            "w0": w0.reshape(1, HID).astype(_bf),
            "msum": msum[lo:hi].reshape(1, VH).astype(_bf),
        }
        in_maps.append(m)
    return shared, in_maps


# ---------------------------------------------------------------------------
# Device program
# ---------------------------------------------------------------------------

_PROG_CACHE = {}


def _build_program(has_gnb, colb_idx, ncolb):
    import contextlib

    import concourse.tile as tile
    from concourse import mybir, bacc

    bf16 = mybir.dt.bfloat16
    f32 = mybir.dt.float32
    AF = mybir.ActivationFunctionType
    OP = mybir.AluOpType

    nc = bacc.Bacc("TRN2", target_bir_lowering=False, debug=False, num_devices=NCORES)

    def inp(name, shape, dt_):
        return nc.declare_dram_parameter(name, shape, dt_, isOutput=False)

    h0_d = inp("h0", [V, F0], bf16)
    mt_d = inp("mt", [V, VH], bf16)
    xskip_d = inp("xskip", [VH, HID], f32)
    cond_d = inp("cond", [NL, HID, VH], f32)
    wg0_d = inp("wg0", [F0, HID], bf16)
    wg12_d = inp("wg12", [2, HID, HID], bf16)
    wq_d = inp("wq", [NL, HID, HID], bf16)
    wk_d = inp("wk", [NL, HID, HID], bf16)
    wv_d = inp("wv", [NL, HID, HID], bf16)
    wo_d = inp("wo", [NL, HID, HID], bf16)
    wl_d = inp("wl", [NL, HID, HID], bf16)
    wf1_d = inp("wf1", [NL, HID, FF], bf16)
    wf2_d = inp("wf2", [NL, FF, HID], bf16)
    colb_d = inp("colb", [ncolb, 128], f32)
    natb_d = inp("natb", [2 * NL, 128, HID], f32)
    w0_d = inp("w0", [1, HID], bf16)
    msum_d = inp("msum", [1, VH], bf16)
    out_d = nc.declare_dram_parameter("out", [VH, HID], f32, isOutput=True)

    kv_in = [nc.dram_tensor(f"kv_in{l}", [KVFLAT], bf16) for l in range(NL)]
    kv_out = [nc.dram_tensor(f"kv_out{l}", [2, KVFLAT], bf16) for l in range(NL)]
    hg_in = [None] + [nc.dram_tensor(f"hg_in{l}", [VH, HID], bf16) for l in (1, 2)]
    hg_out = [None] + [nc.dram_tensor(f"hg_out{l}", [2 * VH, HID], bf16) for l in (1, 2)]

    PAIRS = [[0, 1], [2, 3], [4, 5], [6, 7]]
    SCALE = float(1.0 / np.sqrt(DH))

    with tile.TileContext(nc) as tc, contextlib.ExitStack() as ctx:
        cp = ctx.enter_context(tc.tile_pool(name="const", bufs=1))
        wp = ctx.enter_context(tc.tile_pool(name="work", bufs=1))
        sp = ctx.enter_context(tc.tile_pool(name="small", bufs=1))
        rp = ctx.enter_context(tc.tile_pool(name="rows", bufs=2))
        esp = ctx.enter_context(tc.tile_pool(name="esp", bufs=3))
        pm = ctx.enter_context(tc.tile_pool(name="pm", bufs=2, space="PSUM"))
        psc = ctx.enter_context(tc.tile_pool(name="psc", bufs=2, space="PSUM"))
        pox = ctx.enter_context(tc.tile_pool(name="pox", bufs=1, space="PSUM"))

        # ---------------- persistent loads ----------------
        mt_sb = cp.tile([128, ST, VH], bf16)
        nc.sync.dma_start(mt_sb[:], mt_d.rearrange("(t p) d -> p t d", p=128))
        colb_sb = cp.tile([128, ncolb], f32)
        nc.sync.dma_start(colb_sb[:], colb_d.rearrange("n p -> p n"))
        natb_sb = cp.tile([128, 2 * NL, HID], f32)
        nc.sync.dma_start(natb_sb[:], natb_d.rearrange("n p f -> p n f"))

        wg0_sb = cp.tile([128, 3, HID], bf16)
        nc.vector.memset(wg0_sb[:], 0.0)
        nc.sync.dma_start(
            wg0_sb[:, 0:2, :], wg0_d[0:256].rearrange("(t p) f -> p t f", p=128)
        )
        nc.sync.dma_start(wg0_sb[:64, 2, :], wg0_d[256:320])

        w0_sb = cp.tile([1, HID], bf16)
        nc.sync.dma_start(w0_sb[:], w0_d[:])
        msum_sb = cp.tile([1, VH], bf16)
        nc.sync.dma_start(msum_sb[:], msum_d[:])

        ones_col = cp.tile([128, 1], bf16)
        nc.vector.memset(ones_col[:], 1.0)

        kTf_sb = cp.tile([128, 2, V], bf16)                 # gathered K^T
        vext_sb = cp.tile([128, ST, HEADS, DH + 1], bf16)   # gathered V nat + ones col
        nc.vector.memset(vext_sb[:], 1.0)

        def col(name, j):
            return colb_sb[:, colb_idx[name] + j : colb_idx[name] + j + 1]

        # h0 shares the full-V slot used by the gathered h of later layers
        hfull = cp.tile([128, ST, F0], bf16, tag="hfullg")
        nc.sync.dma_start(hfull[:], h0_d.rearrange("(t p) f -> p t f", p=128))
        hf_tiles = 3

        for l in range(NL):
            final = l == NL - 1

            # ---- per-layer weights ----
            wg_l = wg0_sb
            if l > 0:
                wg_l = cp.tile([128, 3, HID], bf16, tag="wg_l")
                nc.sync.dma_start(
                    wg_l[:, 0:2, :],
                    wg12_d[l - 1].rearrange("(t p) f -> p t f", p=128),
                )

            def loadw(d, kt, fo, tag):
                sb = cp.tile([128, kt, fo], bf16, tag=tag)
                nc.sync.dma_start(sb[:], d[l].rearrange("(t p) f -> p t f", p=128))
                return sb

            wq_sb = loadw(wq_d, 2, HID, "wq")
            wk_sb = loadw(wk_d, 2, HID, "wk")
            wv_sb = loadw(wv_d, 2, HID, "wv")
            wo_sb = loadw(wo_d, 2, HID, "wo")
            wl_sb = loadw(wl_d, 2, HID, "wl")
            wf1_sb = loadw(wf1_d, 2, FF, "wf1")
            wf2_sb = loadw(wf2_d, 8, HID, "wf2")
            cond_sb = cp.tile([128, 2, VH], f32, tag="cond_l")
            nc.sync.dma_start(
                cond_sb[:], cond_d[l].rearrange("(u p) v -> p u v", p=128)
            )

            # ---- gnorm (layer 1): scalar affine applied in place ----
            if l == 1:
                hsq = wp.tile([128, ST, HID], bf16, tag="hres")
                nc.vector.tensor_tensor(hsq[:], hfull[:, :, 0:HID], hfull[:, :, 0:HID], OP.mult)
                s_ps = pm.tile([128, 512], f32, tag="mm")
                q_ps = pm.tile([128, 512], f32, tag="mm")
                for st in range(ST):
                    nc.tensor.matmul(
                        s_ps[0:1, 0:HID], ones_col[:], hfull[:, st, 0:HID],
                        start=(st == 0), stop=(st == ST - 1),
                    )
                for st in range(ST):
                    nc.tensor.matmul(
                        q_ps[0:1, 0:HID], ones_col[:], hsq[:, st, :],
                        start=(st == 0), stop=(st == ST - 1),
                    )
                gn = rp.tile([1, 8], f32, tag="gn")
                nc.vector.reduce_sum(gn[:, 0:1], s_ps[0:1, 0:HID], mybir.AxisListType.X)
                nc.vector.reduce_sum(gn[:, 1:2], q_ps[0:1, 0:HID], mybir.AxisListType.X)
                VC = float(V * HID)
                nc.vector.tensor_scalar_mul(gn[:, 0:1], gn[:, 0:1], 1.0 / VC)  # mu
                nc.vector.tensor_scalar_mul(gn[:, 1:2], gn[:, 1:2], 1.0 / VC)  # E[x^2]
                nc.vector.tensor_tensor(gn[:, 2:3], gn[:, 0:1], gn[:, 0:1], OP.mult)
                nc.vector.tensor_tensor(gn[:, 2:3], gn[:, 1:2], gn[:, 2:3], OP.subtract)
                nc.vector.tensor_scalar_add(gn[:, 2:3], gn[:, 2:3], 1e-5)  # var+eps
                nc.scalar.activation(gn[:, 3:4], gn[:, 2:3], AF.Ln)
                nc.scalar.activation(gn[:, 4:5], gn[:, 3:4], AF.Exp, scale=-0.5)
                mu_c = sp.tile([128, 1], f32, tag="gmu")
                rs_c = sp.tile([128, 1], f32, tag="grs")
                nc.gpsimd.partition_broadcast(mu_c[:], gn[:, 0:1])
                nc.gpsimd.partition_broadcast(rs_c[:], gn[:, 4:5])
                nc.vector.tensor_scalar(
                    out=hfull[:, :, 0:HID], in0=hfull[:, :, 0:HID],
                    scalar1=mu_c[:], scalar2=rs_c[:],
                    op0=OP.subtract, op1=OP.mult,
                )

            # ---- gcn: zT = (M @ h)^T, then yT = (z @ Wg)^T ----
            zT = wp.tile([128, 3, VH], bf16, tag="zT")
            for ft in range(hf_tiles):
                fw = 64 if (l == 0 and ft == 2) else 128
                for qc in range(QC):
                    z_ps = pm.tile([128, 512], f32, tag="mm")
                    for st in range(ST):
                        nc.tensor.matmul(
                            z_ps[0:fw, :],
                            hfull[:, st, ft * 128 : ft * 128 + fw],
                            mt_sb[:, st, qc * 512 : (qc + 1) * 512],
                            start=(st == 0), stop=(st == ST - 1),
                        )
                    nc.vector.tensor_copy(
                        zT[0:fw, ft, qc * 512 : (qc + 1) * 512], z_ps[0:fw, :]
                    )

            gnb_term = has_gnb and l == 1
            hres = wp.tile([128, 2, VH], f32, tag="hres")
            hbf = wp.tile([128, 2, VH], bf16, tag="hbf")
            for u in range(2):
                for qc in range(QC):
                    y_ps = pm.tile([128, 512], f32, tag="mm")
                    for ft in range(hf_tiles):
                        fw = 64 if (l == 0 and ft == 2) else 128
                        nc.tensor.matmul(
                            y_ps[:],
                            wg_l[0:fw, ft, u * 128 : (u + 1) * 128],
                            zT[0:fw, ft, qc * 512 : (qc + 1) * 512],
                            start=(ft == 0),
                            stop=(ft == hf_tiles - 1 and not gnb_term),
                        )
                    if gnb_term:
                        nc.tensor.matmul(
                            y_ps[:],
                            w0_sb[:, u * 128 : (u + 1) * 128],
                            msum_sb[:, qc * 512 : (qc + 1) * 512],
                            start=False, stop=True,
                        )
                    qs = slice(qc * 512, (qc + 1) * 512)
                    if not final:
                        nc.vector.tensor_scalar(
                            out=hres[:, u, qs], in0=y_ps[:], scalar1=col(f"bg{l}", u),
                            scalar2=0.0, op0=OP.add, op1=OP.max,
                        )
                    else:
                        nc.vector.tensor_scalar(
                            out=hres[:, u, qs], in0=y_ps[:], scalar1=col(f"bg{l}", u),
                            scalar2=col("gm0", u), op0=OP.add, op1=OP.mult,
                        )
                        nc.vector.tensor_scalar(
                            out=hres[:, u, qs], in0=hres[:, u, qs],
                            scalar1=col("bt0", u), scalar2=0.0,
                            op0=OP.add, op1=OP.max,
                        )
                    nc.vector.tensor_tensor(
                        hres[:, u, qs], hres[:, u, qs], cond_sb[:, u, qs], OP.add
                    )
                    nc.vector.tensor_copy(hbf[:, u, qs], hres[:, u, qs])

            # ---- k, v projections + pair allgather ----
            kT_own = wp.tile([128, 2, VH], bf16, tag="kT_own")
            for u in range(2):
                for qc in range(QC):
                    k_ps = pm.tile([128, 512], f32, tag="mm")
                    for kt in range(2):
                        nc.tensor.matmul(
                            k_ps[:],
                            wk_sb[:, kt, u * 128 : (u + 1) * 128],
                            hbf[:, kt, qc * 512 : (qc + 1) * 512],
                            start=(kt == 0), stop=(kt == 1),
                        )
                    nc.vector.tensor_scalar(
                        out=kT_own[:, u, qc * 512 : (qc + 1) * 512], in0=k_ps[:],
                        scalar1=col(f"bk{l}", u), scalar2=None, op0=OP.add,
                    )
            vnat = wp.tile([128, VS, HID], bf16, tag="vnat")
            for vs in range(VS):
                v_ps = pm.tile([128, 512], f32, tag="mm")
                for kt in range(2):
                    nc.tensor.matmul(
                        v_ps[:, 0:HID],
                        hbf[:, kt, vs * 128 : (vs + 1) * 128],
                        wv_sb[:, kt, :],
                        start=(kt == 0), stop=(kt == 1),
                    )
                nc.vector.tensor_tensor(
                    vnat[:, vs, :], v_ps[:, 0:HID], natb_sb[:, l, :], OP.add
                )
            nc.sync.dma_start(
                kv_in[l][0 : HID * VH].rearrange("(p u v) -> p u v", p=128, u=2),
                kT_own[:],
            )
            nc.sync.dma_start(
                kv_in[l][HID * VH :].rearrange("(t p) f -> p t f", p=128)
                if False else
                kv_in[l][HID * VH :].rearrange("(t p f) -> p t f", p=128, f=HID),
                vnat[:],
            )
            nc.gpsimd.collective_compute(
                "AllGather", mybir.AluOpType.bypass,
                replica_groups=PAIRS,
                ins=[kv_in[l][:].opt()], outs=[kv_out[l][:].opt()],
            )
            for g in range(2):
                nc.gpsimd.dma_start(
                    kTf_sb[:, :, g * VH : (g + 1) * VH],
                    kv_out[l][g, 0 : HID * VH].rearrange(
                        "(p u v) -> p u v", p=128, u=2
                    ),
                )
                for st8 in range(8):
                    nc.gpsimd.dma_start(
                        vext_sb[:, 8 * g + st8, :, 0:DH],
                        kv_out[l][
                            g,
                            HID * VH + st8 * 128 * HID : HID * VH + (st8 + 1) * 128 * HID,
                        ].rearrange("(p h d) -> p h d", p=128, h=HEADS),
                    )

            # ---- q projection ----
            qT = wp.tile([128, 2, VH], bf16, tag="qT")
            for u in range(2):
                for qc in range(QC):
                    q_ps = pm.tile([128, 512], f32, tag="mm")
                    for kt in range(2):
                        nc.tensor.matmul(
                            q_ps[:],
                            wq_sb[:, kt, u * 128 : (u + 1) * 128],
                            hbf[:, kt, qc * 512 : (qc + 1) * 512],
                            start=(kt == 0), stop=(kt == 1),
                        )
                    nc.vector.tensor_scalar(
                        out=qT[:, u, qc * 512 : (qc + 1) * 512], in0=q_ps[:],
                        scalar1=col(f"bq{l}", u), scalar2=None, op0=OP.add,
                    )

            # ---- attention + per-q-chunk tail pipeline ----
            oT = wp.tile([128, 2, VH], bf16, tag="oT")
            if not final:
                hnat_bf = wp.tile([128, VS, HID], bf16, tag="hnatb")
            for qc in range(QC):
                qs = slice(qc * 512, (qc + 1) * 512)
                for hg in range(4):          # head pairs
                    u = hg // 2
                    hh0 = (hg % 2) * 2
                    o_ps = pox.tile([DH + 1, 2, 512], f32, tag="oext")
                    for st in range(ST):
                        s_ps = psc.tile([128, 2, 512], f32, tag="sc")
                        for j in range(2):
                            hh = hh0 + j
                            nc.tensor.matmul(
                                s_ps[:, j, :],
                                kTf_sb[32 * hh : 32 * hh + 32, u,
                                       st * 128 : (st + 1) * 128],
                                qT[32 * hh : 32 * hh + 32, u, qs],
                                start=True, stop=True,
                                tile_position=(32 * hh, 0),
                            )
                        es = esp.tile([128, 2, 512], bf16, tag="es")
                        nc.scalar.activation(es[:], s_ps[:], AF.Exp, scale=SCALE)
                        for j in range(2):
                            h_abs = u * 4 + hh0 + j
                            nc.tensor.matmul(
                                o_ps[:, j, :],
                                vext_sb[:, st, h_abs, :],
                                es[:, j, :],
                                start=(st == 0), stop=(st == ST - 1),
                            )
                    srow = rp.tile([1, 2, 512], f32, tag="avs")
                    nc.vector.tensor_copy(srow[:], o_ps[DH : DH + 1, :, :])
                    sbc = rp.tile([DH, 2, 512], f32, tag="avb")
                    nc.gpsimd.partition_broadcast(sbc[:], srow[:])
                    nc.vector.reciprocal(sbc[:], sbc[:])
                    for j in range(2):
                        hh = hh0 + j
                        nc.vector.tensor_tensor(
                            oT[32 * hh : 32 * hh + 32, u, qs],
                            o_ps[0:DH, j, :], sbc[:, j, :], OP.mult,
                        )

                # ---- h = h + o @ Wo + bo (this q-chunk) ----
                for u in range(2):
                    w_ps = pm.tile([128, 512], f32, tag="mm")
                    for kt in range(2):
                        nc.tensor.matmul(
                            w_ps[:],
                            wo_sb[:, kt, u * 128 : (u + 1) * 128],
                            oT[:, kt, qs],
                            start=(kt == 0), stop=(kt == 1),
                        )
                    tmp = sp.tile([128, 512], f32, tag="wotmp")
                    nc.vector.tensor_scalar(
                        out=tmp[:], in0=w_ps[:], scalar1=col(f"bo{l}", u),
                        scalar2=None, op0=OP.add,
                    )
                    nc.vector.tensor_tensor(
                        hres[:, u, qs], hres[:, u, qs], tmp[:], OP.add
                    )
                    nc.gpsimd.tensor_copy(hbf[:, u, qs], hres[:, u, qs])

                # ---- ff (this q-chunk) ----
                f1 = wp.tile([128, 8, 512], bf16, tag="f1")
                for m in range(8):
                    f_ps = pm.tile([128, 512], f32, tag="mm")
                    for kt in range(2):
                        nc.tensor.matmul(
                            f_ps[:],
                            wf1_sb[:, kt, m * 128 : (m + 1) * 128],
                            hbf[:, kt, qs],
                            start=(kt == 0), stop=(kt == 1),
                        )
                    nc.vector.tensor_scalar(
                        out=f1[:, m, :], in0=f_ps[:],
                        scalar1=col(f"bf1_{l}", m), scalar2=0.0,
                        op0=OP.add, op1=OP.max,
                    )
                for u in range(2):
                    g_ps = pm.tile([128, 512], f32, tag="mm")
                    for kt in range(8):
                        nc.tensor.matmul(
                            g_ps[:],
                            wf2_sb[:, kt, u * 128 : (u + 1) * 128],
                            f1[:, kt, :],
                            start=(kt == 0), stop=(kt == 7),
                        )
                    tmp = sp.tile([128, 512], f32, tag="fftmp")
                    nc.vector.tensor_scalar(
                        out=tmp[:], in0=g_ps[:], scalar1=col(f"bf2_{l}", u),
                        scalar2=None, op0=OP.add,
                    )
                    nc.vector.tensor_tensor(
                        hres[:, u, qs], hres[:, u, qs], tmp[:], OP.add
                    )
                    nc.gpsimd.tensor_copy(hbf[:, u, qs], hres[:, u, qs])

                # ---- layernorm (this q-chunk) ----
                hsq2v = wp.tile([128, 2, 512], bf16, tag="hsq")
                nc.vector.tensor_tensor(
                    hsq2v[:], hbf[:, :, qs], hbf[:, :, qs], OP.mult
                )
                s1 = rp.tile([1, 512], f32, tag="lns1")
                s2 = rp.tile([1, 512], f32, tag="lns2")
                r_ps = pm.tile([128, 512], f32, tag="mm")
                for u in range(2):
                    nc.tensor.matmul(
                        r_ps[0:1, :], ones_col[:], hbf[:, u, qs],
                        start=(u == 0), stop=(u == 1),
                    )
                nc.vector.tensor_scalar_mul(s1[:], r_ps[0:1, :], 1.0 / HID)
                r2_ps = pm.tile([128, 512], f32, tag="mm")
                for u in range(2):
                    nc.tensor.matmul(
                        r2_ps[0:1, :], ones_col[:], hsq2v[:, u, :],
                        start=(u == 0), stop=(u == 1),
                    )
                nc.vector.tensor_scalar_mul(s2[:], r2_ps[0:1, :], 1.0 / HID)
                var = rp.tile([1, 512], f32, tag="lnvar")
                nc.vector.tensor_tensor(var[:], s1[:], s1[:], OP.mult)
                nc.vector.tensor_tensor(var[:], s2[:], var[:], OP.subtract)
                nc.vector.tensor_scalar_add(var[:], var[:], 1e-5)
                lnv = rp.tile([1, 512], f32, tag="lnlog")
                nc.scalar.activation(lnv[:], var[:], AF.Ln)
                rstd = var
                nc.scalar.activation(rstd[:], lnv[:], AF.Exp, scale=-0.5)
                nb = s2
                nc.vector.tensor_tensor(nb[:], s1[:], rstd[:], OP.mult)
                nc.vector.tensor_scalar_mul(nb[:], nb[:], -1.0)
                a_bc = rp.tile([128, 512], f32, tag="lnabc")
                b_bc = rp.tile([128, 512], f32, tag="lnbbc")
                nc.gpsimd.partition_broadcast(a_bc[:], rstd[:])
                nc.gpsimd.partition_broadcast(b_bc[:], nb[:])
                hln = wp.tile([128, 2, VH], bf16, tag="hln")
                for u in range(2):
                    nc.vector.tensor_tensor(
                        hres[:, u, qs], hres[:, u, qs], a_bc[:], OP.mult
                    )
                    nc.vector.tensor_tensor(
                        hres[:, u, qs], hres[:, u, qs], b_bc[:], OP.add
                    )
                    nc.vector.tensor_scalar(
                        out=hln[:, u, qs], in0=hres[:, u, qs],
                        scalar1=col(f"lng{l}", u),
                        scalar2=col(f"lnb{l}", u), op0=OP.mult, op1=OP.add,
                    )

                # ---- Wl linear (this q-chunk's v-slices) ----
                for vi in range(4):
                    vs = qc * 4 + vi
                    n_ps = pm.tile([128, 512], f32, tag="mm")
                    for kt in range(2):
                        nc.tensor.matmul(
                            n_ps[:, 0:HID],
                            hln[:, kt, vs * 128 : (vs + 1) * 128],
                            wl_sb[:, kt, :],
                            start=(kt == 0), stop=(kt == 1),
                        )
                    if not final:
                        tmp = sp.tile([128, HID], f32, tag="wltmp")
                        nc.vector.tensor_tensor(
                            tmp[:], n_ps[:, 0:HID], natb_sb[:, NL + l, :], OP.add
                        )
                        nc.vector.tensor_scalar(
                            out=hnat_bf[:, vs, :], in0=tmp[:], scalar1=0.0,
                            scalar2=None, op0=OP.max,
                        )
                    else:
                        xs = sp.tile([128, HID], f32, tag="xstmp")
                        nc.sync.dma_start(
                            xs[:], xskip_d[vs * 128 : (vs + 1) * 128, :]
                        )
                        tmp = sp.tile([128, HID], f32, tag="wltmp")
                        nc.vector.tensor_tensor(
                            tmp[:], n_ps[:, 0:HID], natb_sb[:, NL + l, :], OP.add
                        )
                        out2 = sp.tile([128, HID], f32, tag="outtmp")
                        nc.vector.tensor_tensor(out2[:], tmp[:], xs[:], OP.add)
                        nc.sync.dma_start(
                            out_d[vs * 128 : (vs + 1) * 128, :], out2[:]
                        )
                if not final:
                    nc.sync.dma_start(
                        hg_in[l + 1][qc * 512 : (qc + 1) * 512, :].rearrange(
                            "(t p) f -> p t f", p=128
                        ),
                        hnat_bf[:, qc * 4 : qc * 4 + 4, :],
                    )

            if not final:
                nc.gpsimd.collective_compute(
                    "AllGather", mybir.AluOpType.bypass,
                    replica_groups=PAIRS,
                    ins=[hg_in[l + 1][:].opt()], outs=[hg_out[l + 1][:].opt()],
                )
                hfull = cp.tile([128, ST, F0], bf16, tag="hfullg")
                nc.gpsimd.dma_start(
                    hfull[:, :, 0:HID],
                    hg_out[l + 1].rearrange("(t p) f -> p t f", p=128),
                )
                hf_tiles = 2

    nc.finalize()
    return nc


LAST_EXEC_NS = None


def kernel(x, cond_x, edge_index, t, params):
    import os
    from concourse.bass_utils import run_bass_kernel_spmd

    shared, in_maps = _host_prep(x, cond_x, edge_index, t, params)

    key = (shared["has_gnb"],)
    if key not in _PROG_CACHE:
        _PROG_CACHE[key] = _build_program(
            shared["has_gnb"], shared["colb_idx"], shared["ncolb"]
        )
    nc = _PROG_CACHE[key]

    trace = os.environ.get("ATTGNN_TRACE", "0") == "1"
    r = run_bass_kernel_spmd(nc, in_maps, list(range(NCORES)), trace=trace)
    global LAST_EXEC_NS
    LAST_EXEC_NS = r.exec_time_ns
    out = np.zeros((B, V, HID), np.float32)
    for core in range(NCORES):
        b, rr = core // 2, core % 2
        out[b, rr * VH : (rr + 1) * VH] = r.results[core]["out"]
    return out


# revision 11
# speedup vs baseline: 1.1160x; 1.1160x over previous
"""Trainium2 Bass kernel for nn_AttGNN (3-layer GCN+attention network).

Sharding: 8 NeuronCores = 4 batch elements x 2-way node split (each core
owns 1024 of the 2048 nodes). Pair AllGathers supply the full node set
where needed (GCN aggregation input, attention K/V).

GCN message passing is reformulated as a dense matmul against the
host-precomputed symmetric-normalized adjacency (self-loop folded into
the diagonal), reordered as (M @ h) @ Wg so only the natural-layout h is
needed. The GroupNorm per-channel gain is folded into Wg on the host.
"""

import sys

sys.path.insert(0, "/opt/trn_rl_repo")

import numpy as np
import ml_dtypes

B, V, E = 4, 2048, 65536
F_IN, HID, COND, ENC, HEADS, NL = 256, 256, 64, 128, 8, 3
FF = 4 * HID
VH = V // 2          # nodes owned per core
DH = HID // HEADS    # 32
F0 = F_IN + COND     # 320, first-layer input width
NCORES = 8
KVFLAT = HID * VH + VH * HID   # flattened kT + v_nat halves, in elements
ST = V // 128        # 16 s-tiles over full V
VS = VH // 128       # 8 v-slices over own nodes
QC = VH // 512       # 2 q-chunks of 512

_bf = ml_dtypes.bfloat16


def _np(a):
    return np.asarray(a)


def _host_prep(x, cond_x, edge_index, t, params):
    """Build the per-core input maps (all numpy)."""
    x = _np(x).astype(np.float32)
    cond_x = _np(cond_x).astype(np.float32)
    ei = _np(edge_index).astype(np.int64)
    t = _np(t).astype(np.float32)

    src, dst = ei[0], ei[1]
    deg = np.zeros(V, np.float32)
    np.add.at(deg, dst, np.float32(1.0))
    deg += 1.0
    dinv = (1.0 / np.sqrt(deg)).astype(np.float32)

    # MT[s, d] = A[d, s]; A = normalized adjacency + diag(1/deg)
    MT = np.zeros((V, V), np.float32)
    np.add.at(MT, (src, dst), dinv[src] * dinv[dst])
    MT[np.arange(V), np.arange(V)] += 1.0 / deg
    msum = MT.sum(axis=0)  # row sums of A (for optional gn_b term)

    gn_g = _np(params["gn_g"]).astype(np.float32)
    gn_b = _np(params["gn_b"]).astype(np.float32)
    Wfilm = _np(params["Wfilm"]).astype(np.float32)
    bfilm = _np(params["bfilm"]).astype(np.float32)

    film = t @ Wfilm + bfilm          # [B, 2*HID]
    gm, bt = film[:, :HID], film[:, HID:]

    L = [
        {k: _np(v).astype(np.float32) for k, v in params["layers"][i].items()}
        for i in range(NL)
    ]

    # fold gn_g into Wg of layer 1 (the only gnorm'ed gcn input)
    Wg_eff = [L[0]["Wg"], gn_g[:, None] * L[1]["Wg"], L[2]["Wg"]]
    w0 = gn_b @ L[1]["Wg"]            # [HID]; nonzero only if gn_b != 0
    has_gnb = bool(np.any(gn_b != 0.0))

    condT = np.stack(
        [(cond_x @ L[i]["We"] + L[i]["be"]).T.astype(np.float32) for i in range(NL)]
    )  # [NL, HID, V]

    rows = []
    colb_idx = {}

    def add_col(name, vec):
        vec = vec.reshape(-1, 128)
        colb_idx[name] = len(rows)
        rows.extend(list(vec))

    for i in range(NL):
        add_col(f"bg{i}", L[i]["bg"])
        add_col(f"bq{i}", L[i]["bq"])
        add_col(f"bk{i}", L[i]["bk"])
        add_col(f"bo{i}", L[i]["bo"])
        add_col(f"bf2_{i}", L[i]["bf2"])
        add_col(f"lng{i}", L[i]["ln_g"])
        add_col(f"lnb{i}", L[i]["ln_b"])
        add_col(f"bf1_{i}", L[i]["bf1"])
    add_col("gm0", gm[0])   # per-core batch values patched in kernel()
    add_col("bt0", bt[0])
    colb = np.stack(rows).astype(np.float32)

    natb = np.stack(
        [np.broadcast_to(L[i]["bv"], (128, HID)) for i in range(NL)]
        + [np.broadcast_to(L[i]["bl"], (128, HID)) for i in range(NL)]
    ).astype(np.float32)

    wg0 = Wg_eff[0].astype(_bf)
    wg12 = np.stack([Wg_eff[1], Wg_eff[2]]).astype(_bf)
    wq = np.stack([L[i]["Wq"] for i in range(NL)]).astype(_bf)
    wk = np.stack([L[i]["Wk"] for i in range(NL)]).astype(_bf)
    wv = np.stack([L[i]["Wv"] for i in range(NL)]).astype(_bf)
    wo = np.stack([L[i]["Wo"] for i in range(NL)]).astype(_bf)
    wl = np.stack([L[i]["Wl"] for i in range(NL)]).astype(_bf)
    wf1 = np.stack([L[i]["Wf1"] for i in range(NL)]).astype(_bf)
    wf2 = np.stack([L[i]["Wf2"] for i in range(NL)]).astype(_bf)

    shared = {
        "colb_idx": colb_idx,
        "has_gnb": has_gnb,
        "gm": gm,
        "bt": bt,
        "ncolb": colb.shape[0],
    }

    in_maps = []
    for core in range(NCORES):
        b, r = core // 2, core % 2
        lo, hi = r * VH, (r + 1) * VH
        cb = colb.copy()
        cb[colb_idx["gm0"] : colb_idx["gm0"] + 2] = gm[b].reshape(2, 128)
        cb[colb_idx["bt0"] : colb_idx["bt0"] + 2] = bt[b].reshape(2, 128)
        m = {
            "h0": np.concatenate([x[b], cond_x], axis=-1).astype(_bf),
            "mt": MT[:, lo:hi].astype(_bf),
            "xskip": x[b, lo:hi].astype(np.float32),
            "cond": condT[:, :, lo:hi].astype(np.float32),
            "wg0": wg0,
            "wg12": wg12,
            "wq": wq,
            "wk": wk,
            "wv": wv,
            "wo": wo,
            "wl": wl,
            "wf1": wf1,
            "wf2": wf2,
            "colb": cb,
            "natb": natb,
            "rowb": np.concatenate(
                [L[i]["bo"] for i in range(NL)] + [L[i]["bf2"] for i in range(NL)]
            ).reshape(1, -1).astype(_bf),
            "w0": w0.reshape(1, HID).astype(_bf),
            "msum": msum[lo:hi].reshape(1, VH).astype(_bf),
        }
        in_maps.append(m)
    return shared, in_maps


# ---------------------------------------------------------------------------
# Device program
# ---------------------------------------------------------------------------

_PROG_CACHE = {}


def _build_program(has_gnb, colb_idx, ncolb):
    import contextlib

    import concourse.tile as tile
    from concourse import mybir, bacc

    bf16 = mybir.dt.bfloat16
    f32 = mybir.dt.float32
    AF = mybir.ActivationFunctionType
    OP = mybir.AluOpType

    nc = bacc.Bacc("TRN2", target_bir_lowering=False, debug=False, num_devices=NCORES)

    def inp(name, shape, dt_):
        return nc.declare_dram_parameter(name, shape, dt_, isOutput=False)

    h0_d = inp("h0", [V, F0], bf16)
    mt_d = inp("mt", [V, VH], bf16)
    xskip_d = inp("xskip", [VH, HID], f32)
    cond_d = inp("cond", [NL, HID, VH], f32)
    wg0_d = inp("wg0", [F0, HID], bf16)
    wg12_d = inp("wg12", [2, HID, HID], bf16)
    wq_d = inp("wq", [NL, HID, HID], bf16)
    wk_d = inp("wk", [NL, HID, HID], bf16)
    wv_d = inp("wv", [NL, HID, HID], bf16)
    wo_d = inp("wo", [NL, HID, HID], bf16)
    wl_d = inp("wl", [NL, HID, HID], bf16)
    wf1_d = inp("wf1", [NL, HID, FF], bf16)
    wf2_d = inp("wf2", [NL, FF, HID], bf16)
    colb_d = inp("colb", [ncolb, 128], f32)
    natb_d = inp("natb", [2 * NL, 128, HID], f32)
    rowb_d = inp("rowb", [1, 12 * 128], bf16)
    w0_d = inp("w0", [1, HID], bf16)
    msum_d = inp("msum", [1, VH], bf16)
    out_d = nc.declare_dram_parameter("out", [VH, HID], f32, isOutput=True)

    kv_in = [nc.dram_tensor(f"kv_in{l}", [KVFLAT], bf16) for l in range(NL)]
    kv_out = [nc.dram_tensor(f"kv_out{l}", [2, KVFLAT], bf16) for l in range(NL)]
    hg_in = [None] + [nc.dram_tensor(f"hg_in{l}", [VH, HID], bf16) for l in (1, 2)]
    hg_out = [None] + [nc.dram_tensor(f"hg_out{l}", [2 * VH, HID], bf16) for l in (1, 2)]

    PAIRS = [[0, 1], [2, 3], [4, 5], [6, 7]]
    SCALE = float(1.0 / np.sqrt(DH))

    with tile.TileContext(nc) as tc, contextlib.ExitStack() as ctx:
        cp = ctx.enter_context(tc.tile_pool(name="const", bufs=1))
        wp = ctx.enter_context(tc.tile_pool(name="work", bufs=1))
        sp = ctx.enter_context(tc.tile_pool(name="small", bufs=1))
        rp = ctx.enter_context(tc.tile_pool(name="rows", bufs=2))
        esp = ctx.enter_context(tc.tile_pool(name="esp", bufs=3))
        pm = ctx.enter_context(tc.tile_pool(name="pm", bufs=2, space="PSUM"))
        psc = ctx.enter_context(tc.tile_pool(name="psc", bufs=2, space="PSUM"))
        pox = ctx.enter_context(tc.tile_pool(name="pox", bufs=1, space="PSUM"))

        # ---------------- persistent loads ----------------
        mt_sb = cp.tile([128, ST, VH], bf16)
        nc.sync.dma_start(mt_sb[:], mt_d.rearrange("(t p) d -> p t d", p=128))
        colb_sb = cp.tile([128, ncolb], f32)
        nc.sync.dma_start(colb_sb[:], colb_d.rearrange("n p -> p n"))
        natb_sb = cp.tile([128, 2 * NL, HID], f32)
        nc.sync.dma_start(natb_sb[:], natb_d.rearrange("n p f -> p n f"))

        wg0_sb = cp.tile([128, 3, HID], bf16)
        nc.vector.memset(wg0_sb[:], 0.0)
        nc.sync.dma_start(
            wg0_sb[:, 0:2, :], wg0_d[0:256].rearrange("(t p) f -> p t f", p=128)
        )
        nc.sync.dma_start(wg0_sb[:64, 2, :], wg0_d[256:320])

        rowb_sb = cp.tile([1, 12 * 128], bf16)
        nc.sync.dma_start(rowb_sb[:], rowb_d[:])
        ones_row = cp.tile([1, 512], bf16)
        nc.vector.memset(ones_row[:], 1.0)

        def rowb(name, j):
            l_ = int(name[-1])
            base = 0 if name.startswith("bo") else 6
            r = base + 2 * l_ + j
            return rowb_sb[0:1, r * 128 : (r + 1) * 128]

        w0_sb = cp.tile([1, HID], bf16)
        nc.sync.dma_start(w0_sb[:], w0_d[:])
        msum_sb = cp.tile([1, VH], bf16)
        nc.sync.dma_start(msum_sb[:], msum_d[:])

        ones_col = cp.tile([128, 1], bf16)
        nc.vector.memset(ones_col[:], 1.0)

        kTf_sb = cp.tile([128, 2, V], bf16)                 # gathered K^T
        vext_sb = cp.tile([128, ST, HEADS, DH + 1], bf16)   # gathered V nat + ones col
        nc.vector.memset(vext_sb[:], 1.0)

        def col(name, j):
            return colb_sb[:, colb_idx[name] + j : colb_idx[name] + j + 1]

        # h0 shares the full-V slot used by the gathered h of later layers
        hfull = cp.tile([128, ST, F0], bf16, tag="hfullg")
        nc.sync.dma_start(hfull[:], h0_d.rearrange("(t p) f -> p t f", p=128))
        hf_tiles = 3

        for l in range(NL):
            final = l == NL - 1

            # ---- per-layer weights ----
            wg_l = wg0_sb
            if l > 0:
                wg_l = cp.tile([128, 3, HID], bf16, tag="wg_l")
                nc.sync.dma_start(
                    wg_l[:, 0:2, :],
                    wg12_d[l - 1].rearrange("(t p) f -> p t f", p=128),
                )

            def loadw(d, kt, fo, tag):
                sb = cp.tile([128, kt, fo], bf16, tag=tag)
                nc.sync.dma_start(sb[:], d[l].rearrange("(t p) f -> p t f", p=128))
                return sb

            wq_sb = loadw(wq_d, 2, HID, "wq")
            wk_sb = loadw(wk_d, 2, HID, "wk")
            wv_sb = loadw(wv_d, 2, HID, "wv")
            wo_sb = loadw(wo_d, 2, HID, "wo")
            wl_sb = loadw(wl_d, 2, HID, "wl")
            wf1_sb = loadw(wf1_d, 2, FF, "wf1")
            wf2_sb = loadw(wf2_d, 8, HID, "wf2")
            cond_sb = cp.tile([128, 2, VH], f32, tag="cond_l")
            nc.sync.dma_start(
                cond_sb[:], cond_d[l].rearrange("(u p) v -> p u v", p=128)
            )

            # ---- gnorm (layer 1): scalar affine applied in place ----
            if l == 1:
                hsq = wp.tile([128, ST, HID], bf16, tag="hres")
                nc.vector.tensor_tensor(hsq[:], hfull[:, :, 0:HID], hfull[:, :, 0:HID], OP.mult)
                s_ps = pm.tile([128, 512], f32, tag="mm")
                q_ps = pm.tile([128, 512], f32, tag="mm")
                for st in range(ST):
                    nc.tensor.matmul(
                        s_ps[0:1, 0:HID], ones_col[:], hfull[:, st, 0:HID],
                        start=(st == 0), stop=(st == ST - 1),
                    )
                for st in range(ST):
                    nc.tensor.matmul(
                        q_ps[0:1, 0:HID], ones_col[:], hsq[:, st, :],
                        start=(st == 0), stop=(st == ST - 1),
                    )
                gn = rp.tile([1, 8], f32, tag="gn")
                nc.vector.reduce_sum(gn[:, 0:1], s_ps[0:1, 0:HID], mybir.AxisListType.X)
                nc.vector.reduce_sum(gn[:, 1:2], q_ps[0:1, 0:HID], mybir.AxisListType.X)
                VC = float(V * HID)
                nc.vector.tensor_scalar_mul(gn[:, 0:1], gn[:, 0:1], 1.0 / VC)  # mu
                nc.vector.tensor_scalar_mul(gn[:, 1:2], gn[:, 1:2], 1.0 / VC)  # E[x^2]
                nc.vector.tensor_tensor(gn[:, 2:3], gn[:, 0:1], gn[:, 0:1], OP.mult)
                nc.vector.tensor_tensor(gn[:, 2:3], gn[:, 1:2], gn[:, 2:3], OP.subtract)
                nc.vector.tensor_scalar_add(gn[:, 2:3], gn[:, 2:3], 1e-5)  # var+eps
                nc.scalar.activation(gn[:, 3:4], gn[:, 2:3], AF.Ln)
                nc.scalar.activation(gn[:, 4:5], gn[:, 3:4], AF.Exp, scale=-0.5)
                mu_c = sp.tile([128, 1], f32, tag="gmu")
                rs_c = sp.tile([128, 1], f32, tag="grs")
                nc.gpsimd.partition_broadcast(mu_c[:], gn[:, 0:1])
                nc.gpsimd.partition_broadcast(rs_c[:], gn[:, 4:5])
                nc.vector.tensor_scalar(
                    out=hfull[:, :, 0:HID], in0=hfull[:, :, 0:HID],
                    scalar1=mu_c[:], scalar2=rs_c[:],
                    op0=OP.subtract, op1=OP.mult,
                )

            # ---- gcn: zT = (M @ h)^T, then yT = (z @ Wg)^T ----
            zT = wp.tile([128, 3, VH], bf16, tag="zT")
            for ft in range(hf_tiles):
                fw = 64 if (l == 0 and ft == 2) else 128
                for qc in range(QC):
                    z_ps = pm.tile([128, 512], f32, tag="mm")
                    for st in range(ST):
                        nc.tensor.matmul(
                            z_ps[0:fw, :],
                            hfull[:, st, ft * 128 : ft * 128 + fw],
                            mt_sb[:, st, qc * 512 : (qc + 1) * 512],
                            start=(st == 0), stop=(st == ST - 1),
                        )
                    nc.vector.tensor_copy(
                        zT[0:fw, ft, qc * 512 : (qc + 1) * 512], z_ps[0:fw, :]
                    )

            gnb_term = has_gnb and l == 1
            hres = wp.tile([128, 2, VH], f32, tag="hres")
            hbf = wp.tile([128, 2, VH], bf16, tag="hbf")
            for u in range(2):
                for qc in range(QC):
                    y_ps = pm.tile([128, 512], f32, tag="mm")
                    for ft in range(hf_tiles):
                        fw = 64 if (l == 0 and ft == 2) else 128
                        nc.tensor.matmul(
                            y_ps[:],
                            wg_l[0:fw, ft, u * 128 : (u + 1) * 128],
                            zT[0:fw, ft, qc * 512 : (qc + 1) * 512],
                            start=(ft == 0),
                            stop=(ft == hf_tiles - 1 and not gnb_term),
                        )
                    if gnb_term:
                        nc.tensor.matmul(
                            y_ps[:],
                            w0_sb[:, u * 128 : (u + 1) * 128],
                            msum_sb[:, qc * 512 : (qc + 1) * 512],
                            start=False, stop=True,
                        )
                    qs = slice(qc * 512, (qc + 1) * 512)
                    if not final:
                        nc.vector.tensor_scalar(
                            out=hres[:, u, qs], in0=y_ps[:], scalar1=col(f"bg{l}", u),
                            scalar2=0.0, op0=OP.add, op1=OP.max,
                        )
                    else:
                        nc.vector.tensor_scalar(
                            out=hres[:, u, qs], in0=y_ps[:], scalar1=col(f"bg{l}", u),
                            scalar2=col("gm0", u), op0=OP.add, op1=OP.mult,
                        )
                        nc.vector.tensor_scalar(
                            out=hres[:, u, qs], in0=hres[:, u, qs],
                            scalar1=col("bt0", u), scalar2=0.0,
                            op0=OP.add, op1=OP.max,
                        )
                    nc.vector.tensor_tensor(
                        hres[:, u, qs], hres[:, u, qs], cond_sb[:, u, qs], OP.add
                    )
                    nc.vector.tensor_copy(hbf[:, u, qs], hres[:, u, qs])

            # ---- k, v projections + pair allgather ----
            kT_own = wp.tile([128, 2, VH], bf16, tag="kT_own")
            for u in range(2):
                for qc in range(QC):
                    k_ps = pm.tile([128, 512], f32, tag="mm")
                    for kt in range(2):
                        nc.tensor.matmul(
                            k_ps[:],
                            wk_sb[:, kt, u * 128 : (u + 1) * 128],
                            hbf[:, kt, qc * 512 : (qc + 1) * 512],
                            start=(kt == 0), stop=(kt == 1),
                        )
                    nc.vector.tensor_scalar(
                        out=kT_own[:, u, qc * 512 : (qc + 1) * 512], in0=k_ps[:],
                        scalar1=col(f"bk{l}", u), scalar2=None, op0=OP.add,
                    )
            vnat = wp.tile([128, VS, HID], bf16, tag="vnat")
            for vs in range(VS):
                v_ps = pm.tile([128, 512], f32, tag="mm")
                for kt in range(2):
                    nc.tensor.matmul(
                        v_ps[:, 0:HID],
                        hbf[:, kt, vs * 128 : (vs + 1) * 128],
                        wv_sb[:, kt, :],
                        start=(kt == 0), stop=(kt == 1),
                    )
                nc.vector.tensor_tensor(
                    vnat[:, vs, :], v_ps[:, 0:HID], natb_sb[:, l, :], OP.add
                )
            nc.sync.dma_start(
                kv_in[l][0 : HID * VH].rearrange("(p u v) -> p u v", p=128, u=2),
                kT_own[:],
            )
            nc.sync.dma_start(
                kv_in[l][HID * VH :].rearrange("(t p) f -> p t f", p=128)
                if False else
                kv_in[l][HID * VH :].rearrange("(t p f) -> p t f", p=128, f=HID),
                vnat[:],
            )
            nc.gpsimd.collective_compute(
                "AllGather", mybir.AluOpType.bypass,
                replica_groups=PAIRS,
                ins=[kv_in[l][:].opt()], outs=[kv_out[l][:].opt()],
            )
            for g in range(2):
                nc.gpsimd.dma_start(
                    kTf_sb[:, :, g * VH : (g + 1) * VH],
                    kv_out[l][g, 0 : HID * VH].rearrange(
                        "(p u v) -> p u v", p=128, u=2
                    ),
                )
                for st8 in range(8):
                    nc.gpsimd.dma_start(
                        vext_sb[:, 8 * g + st8, :, 0:DH],
                        kv_out[l][
                            g,
                            HID * VH + st8 * 128 * HID : HID * VH + (st8 + 1) * 128 * HID,
                        ].rearrange("(p h d) -> p h d", p=128, h=HEADS),
                    )

            # ---- q projection ----
            qT = wp.tile([128, 2, VH], bf16, tag="qT")
            for u in range(2):
                for qc in range(QC):
                    q_ps = pm.tile([128, 512], f32, tag="mm")
                    for kt in range(2):
                        nc.tensor.matmul(
                            q_ps[:],
                            wq_sb[:, kt, u * 128 : (u + 1) * 128],
                            hbf[:, kt, qc * 512 : (qc + 1) * 512],
                            start=(kt == 0), stop=(kt == 1),
                        )
                    nc.vector.tensor_scalar(
                        out=qT[:, u, qc * 512 : (qc + 1) * 512], in0=q_ps[:],
                        scalar1=col(f"bq{l}", u), scalar2=None, op0=OP.add,
                    )

            # ---- attention + per-q-chunk tail pipeline ----
            oT = wp.tile([128, 2, VH], bf16, tag="oT")
            if not final:
                hnat_bf = wp.tile([128, VS, HID], bf16, tag="hnatb")
            for qc in range(QC):
                qs = slice(qc * 512, (qc + 1) * 512)
                for hg in range(4):          # head pairs
                    u = hg // 2
                    hh0 = (hg % 2) * 2
                    o_ps = pox.tile([DH + 1, 2, 512], f32, tag="oext")
                    for st in range(ST):
                        s_ps = psc.tile([128, 2, 512], f32, tag="sc")
                        for j in range(2):
                            hh = hh0 + j
                            nc.tensor.matmul(
                                s_ps[:, j, :],
                                kTf_sb[32 * hh : 32 * hh + 32, u,
                                       st * 128 : (st + 1) * 128],
                                qT[32 * hh : 32 * hh + 32, u, qs],
                                start=True, stop=True,
                                tile_position=(32 * hh, 0),
                            )
                        es = esp.tile([128, 2, 512], bf16, tag="es")
                        nc.scalar.activation(es[:], s_ps[:], AF.Exp, scale=SCALE)
                        for j in range(2):
                            h_abs = u * 4 + hh0 + j
                            nc.tensor.matmul(
                                o_ps[:, j, :],
                                vext_sb[:, st, h_abs, :],
                                es[:, j, :],
                                start=(st == 0), stop=(st == ST - 1),
                            )
                    srow = rp.tile([1, 2, 512], f32, tag="avs")
                    nc.vector.tensor_copy(srow[:], o_ps[DH : DH + 1, :, :])
                    sbc = rp.tile([DH, 2, 512], f32, tag="avb")
                    nc.gpsimd.partition_broadcast(sbc[:], srow[:])
                    nc.vector.reciprocal_approx_fast(out=sbc[:], in_=sbc[:])
                    for j in range(2):
                        hh = hh0 + j
                        nc.vector.tensor_tensor(
                            oT[32 * hh : 32 * hh + 32, u, qs],
                            o_ps[0:DH, j, :], sbc[:, j, :], OP.mult,
                        )

                # ---- h = h + o @ Wo + bo (this q-chunk) ----
                for u in range(2):
                    w_ps = pm.tile([128, 512], f32, tag="mm")
                    for kt in range(2):
                        nc.tensor.matmul(
                            w_ps[:],
                            wo_sb[:, kt, u * 128 : (u + 1) * 128],
                            oT[:, kt, qs],
                            start=(kt == 0), stop=False,
                        )
                    nc.tensor.matmul(
                        w_ps[:], rowb(f"bo{l}", u), ones_row[:],
                        start=False, stop=True,
                    )
                    nc.vector.tensor_tensor(
                        hres[:, u, qs], hres[:, u, qs], w_ps[:], OP.add
                    )
                    nc.vector.tensor_copy(hbf[:, u, qs], hres[:, u, qs])

                # ---- ff (this q-chunk) ----
                f1 = wp.tile([128, 8, 512], bf16, tag="f1")
                for m in range(8):
                    f_ps = pm.tile([128, 512], f32, tag="mm")
                    for kt in range(2):
                        nc.tensor.matmul(
                            f_ps[:],
                            wf1_sb[:, kt, m * 128 : (m + 1) * 128],
                            hbf[:, kt, qs],
                            start=(kt == 0), stop=(kt == 1),
                        )
                    nc.vector.tensor_scalar(
                        out=f1[:, m, :], in0=f_ps[:],
                        scalar1=col(f"bf1_{l}", m), scalar2=0.0,
                        op0=OP.add, op1=OP.max,
                    )
                for u in range(2):
                    g_ps = pm.tile([128, 512], f32, tag="mm")
                    for kt in range(8):
                        nc.tensor.matmul(
                            g_ps[:],
                            wf2_sb[:, kt, u * 128 : (u + 1) * 128],
                            f1[:, kt, :],
                            start=(kt == 0), stop=False,
                        )
                    nc.tensor.matmul(
                        g_ps[:], rowb(f"bf2_{l}", u), ones_row[:],
                        start=False, stop=True,
                    )
                    nc.vector.tensor_tensor(
                        hres[:, u, qs], hres[:, u, qs], g_ps[:], OP.add
                    )
                    nc.vector.tensor_copy(hbf[:, u, qs], hres[:, u, qs])

                # ---- layernorm (this q-chunk) ----
                hsq2v = wp.tile([128, 2, 512], bf16, tag="hsq")
                nc.vector.tensor_tensor(
                    hsq2v[:], hbf[:, :, qs], hbf[:, :, qs], OP.mult
                )
                s1 = rp.tile([1, 512], f32, tag="lns1")
                s2 = rp.tile([1, 512], f32, tag="lns2")
                r_ps = pm.tile([128, 512], f32, tag="mm")
                for u in range(2):
                    nc.tensor.matmul(
                        r_ps[0:1, :], ones_col[:], hbf[:, u, qs],
                        start=(u == 0), stop=(u == 1),
                    )
                nc.vector.tensor_scalar_mul(s1[:], r_ps[0:1, :], 1.0 / HID)
                r2_ps = pm.tile([128, 512], f32, tag="mm")
                for u in range(2):
                    nc.tensor.matmul(
                        r2_ps[0:1, :], ones_col[:], hsq2v[:, u, :],
                        start=(u == 0), stop=(u == 1),
                    )
                nc.vector.tensor_scalar_mul(s2[:], r2_ps[0:1, :], 1.0 / HID)
                var = rp.tile([1, 512], f32, tag="lnvar")
                nc.vector.tensor_tensor(var[:], s1[:], s1[:], OP.mult)
                nc.vector.tensor_tensor(var[:], s2[:], var[:], OP.subtract)
                nc.vector.tensor_scalar_add(var[:], var[:], 1e-5)
                lnv = rp.tile([1, 512], f32, tag="lnlog")
                nc.scalar.activation(lnv[:], var[:], AF.Ln)
                rstd = var
                nc.scalar.activation(rstd[:], lnv[:], AF.Exp, scale=-0.5)
                nb = s2
                nc.vector.tensor_tensor(nb[:], s1[:], rstd[:], OP.mult)
                nc.vector.tensor_scalar_mul(nb[:], nb[:], -1.0)
                a_bc = rp.tile([128, 512], f32, tag="lnabc")
                b_bc = rp.tile([128, 512], f32, tag="lnbbc")
                nc.gpsimd.partition_broadcast(a_bc[:], rstd[:])
                nc.gpsimd.partition_broadcast(b_bc[:], nb[:])
                hln = wp.tile([128, 2, VH], bf16, tag="hln")
                for u in range(2):
                    nc.vector.tensor_tensor(
                        hres[:, u, qs], hres[:, u, qs], a_bc[:], OP.mult
                    )
                    nc.vector.tensor_tensor(
                        hres[:, u, qs], hres[:, u, qs], b_bc[:], OP.add
                    )
                    nc.vector.tensor_scalar(
                        out=hln[:, u, qs], in0=hres[:, u, qs],
                        scalar1=col(f"lng{l}", u),
                        scalar2=col(f"lnb{l}", u), op0=OP.mult, op1=OP.add,
                    )

                # ---- Wl linear (this q-chunk's v-slices) ----
                for vi in range(4):
                    vs = qc * 4 + vi
                    n_ps = pm.tile([128, 512], f32, tag="mm")
                    for kt in range(2):
                        nc.tensor.matmul(
                            n_ps[:, 0:HID],
                            hln[:, kt, vs * 128 : (vs + 1) * 128],
                            wl_sb[:, kt, :],
                            start=(kt == 0), stop=(kt == 1),
                        )
                    if not final:
                        tmp = sp.tile([128, HID], f32, tag="wltmp")
                        nc.vector.tensor_tensor(
                            tmp[:], n_ps[:, 0:HID], natb_sb[:, NL + l, :], OP.add
                        )
                        nc.vector.tensor_scalar(
                            out=hnat_bf[:, vs, :], in0=tmp[:], scalar1=0.0,
                            scalar2=None, op0=OP.max,
                        )
                    else:
                        xs = sp.tile([128, HID], f32, tag="xstmp")
                        nc.sync.dma_start(
                            xs[:], xskip_d[vs * 128 : (vs + 1) * 128, :]
                        )
                        tmp = sp.tile([128, HID], f32, tag="wltmp")
                        nc.vector.tensor_tensor(
                            tmp[:], n_ps[:, 0:HID], natb_sb[:, NL + l, :], OP.add
                        )
                        out2 = sp.tile([128, HID], f32, tag="outtmp")
                        nc.vector.tensor_tensor(out2[:], tmp[:], xs[:], OP.add)
                        nc.sync.dma_start(
                            out_d[vs * 128 : (vs + 1) * 128, :], out2[:]
                        )
                if not final:
                    nc.sync.dma_start(
                        hg_in[l + 1][qc * 512 : (qc + 1) * 512, :].rearrange(
                            "(t p) f -> p t f", p=128
                        ),
                        hnat_bf[:, qc * 4 : qc * 4 + 4, :],
                    )

            if not final:
                nc.gpsimd.collective_compute(
                    "AllGather", mybir.AluOpType.bypass,
                    replica_groups=PAIRS,
                    ins=[hg_in[l + 1][:].opt()], outs=[hg_out[l + 1][:].opt()],
                )
                hfull = cp.tile([128, ST, F0], bf16, tag="hfullg")
                nc.gpsimd.dma_start(
                    hfull[:, :, 0:HID],
                    hg_out[l + 1].rearrange("(t p) f -> p t f", p=128),
                )
                hf_tiles = 2

    nc.finalize()
    return nc


LAST_EXEC_NS = None


def kernel(x, cond_x, edge_index, t, params):
    import os
    from concourse.bass_utils import run_bass_kernel_spmd

    shared, in_maps = _host_prep(x, cond_x, edge_index, t, params)

    key = (shared["has_gnb"],)
    if key not in _PROG_CACHE:
        _PROG_CACHE[key] = _build_program(
            shared["has_gnb"], shared["colb_idx"], shared["ncolb"]
        )
    nc = _PROG_CACHE[key]

    trace = os.environ.get("ATTGNN_TRACE", "0") == "1"
    r = run_bass_kernel_spmd(nc, in_maps, list(range(NCORES)), trace=trace)
    global LAST_EXEC_NS
    LAST_EXEC_NS = r.exec_time_ns
    out = np.zeros((B, V, HID), np.float32)
    for core in range(NCORES):
        b, rr = core // 2, core % 2
        out[b, rr * VH : (rr + 1) * VH] = r.results[core]["out"]
    return out


# revision 13
# speedup vs baseline: 1.1181x; 1.0019x over previous
"""Trainium2 Bass kernel for nn_AttGNN (3-layer GCN+attention network).

Sharding: 8 NeuronCores = 4 batch elements x 2-way node split (each core
owns 1024 of the 2048 nodes). Pair AllGathers supply the full node set
where needed (GCN aggregation input, attention K/V).

GCN message passing is reformulated as a dense matmul against the
host-precomputed symmetric-normalized adjacency (self-loop folded into
the diagonal), reordered as (M @ h) @ Wg so only the natural-layout h is
needed. The GroupNorm per-channel gain is folded into Wg on the host.
"""

import sys

sys.path.insert(0, "/opt/trn_rl_repo")

import numpy as np
import ml_dtypes

B, V, E = 4, 2048, 65536
F_IN, HID, COND, ENC, HEADS, NL = 256, 256, 64, 128, 8, 3
FF = 4 * HID
VH = V // 2          # nodes owned per core
DH = HID // HEADS    # 32
F0 = F_IN + COND     # 320, first-layer input width
NCORES = 8
KVFLAT = HID * VH + VH * HID   # flattened kT + v_nat halves, in elements
ST = V // 128        # 16 s-tiles over full V
VS = VH // 128       # 8 v-slices over own nodes
QC = VH // 512       # 2 q-chunks of 512

_bf = ml_dtypes.bfloat16


def _np(a):
    return np.asarray(a)


def _host_prep(x, cond_x, edge_index, t, params):
    """Build the per-core input maps (all numpy)."""
    x = _np(x).astype(np.float32)
    cond_x = _np(cond_x).astype(np.float32)
    ei = _np(edge_index).astype(np.int64)
    t = _np(t).astype(np.float32)

    src, dst = ei[0], ei[1]
    deg = np.zeros(V, np.float32)
    np.add.at(deg, dst, np.float32(1.0))
    deg += 1.0
    dinv = (1.0 / np.sqrt(deg)).astype(np.float32)

    # MT[s, d] = A[d, s]; A = normalized adjacency + diag(1/deg)
    MT = np.zeros((V, V), np.float32)
    np.add.at(MT, (src, dst), dinv[src] * dinv[dst])
    MT[np.arange(V), np.arange(V)] += 1.0 / deg
    msum = MT.sum(axis=0)  # row sums of A (for optional gn_b term)

    gn_g = _np(params["gn_g"]).astype(np.float32)
    gn_b = _np(params["gn_b"]).astype(np.float32)
    Wfilm = _np(params["Wfilm"]).astype(np.float32)
    bfilm = _np(params["bfilm"]).astype(np.float32)

    film = t @ Wfilm + bfilm          # [B, 2*HID]
    gm, bt = film[:, :HID], film[:, HID:]

    L = [
        {k: _np(v).astype(np.float32) for k, v in params["layers"][i].items()}
        for i in range(NL)
    ]

    # fold gn_g into Wg of layer 1 (the only gnorm'ed gcn input)
    Wg_eff = [L[0]["Wg"], gn_g[:, None] * L[1]["Wg"], L[2]["Wg"]]
    w0 = gn_b @ L[1]["Wg"]            # [HID]; nonzero only if gn_b != 0
    has_gnb = bool(np.any(gn_b != 0.0))

    condT = np.stack(
        [(cond_x @ L[i]["We"] + L[i]["be"]).T.astype(np.float32) for i in range(NL)]
    )  # [NL, HID, V]

    rows = []
    colb_idx = {}

    def add_col(name, vec):
        vec = vec.reshape(-1, 128)
        colb_idx[name] = len(rows)
        rows.extend(list(vec))

    for i in range(NL):
        add_col(f"bg{i}", L[i]["bg"])
        add_col(f"bq{i}", L[i]["bq"])
        add_col(f"bk{i}", L[i]["bk"])
        add_col(f"bo{i}", L[i]["bo"])
        add_col(f"bf2_{i}", L[i]["bf2"])
        add_col(f"lng{i}", L[i]["ln_g"])
        add_col(f"lnb{i}", L[i]["ln_b"])
        add_col(f"bf1_{i}", L[i]["bf1"])
    add_col("gm0", gm[0])   # per-core batch values patched in kernel()
    add_col("bt0", bt[0])
    colb = np.stack(rows).astype(np.float32)

    natb = np.stack(
        [np.broadcast_to(L[i]["bv"], (128, HID)) for i in range(NL)]
        + [np.broadcast_to(L[i]["bl"], (128, HID)) for i in range(NL)]
    ).astype(np.float32)

    wg0 = Wg_eff[0].astype(_bf)
    wg12 = np.stack([Wg_eff[1], Wg_eff[2]]).astype(_bf)
    wq = np.stack([L[i]["Wq"] for i in range(NL)]).astype(_bf)
    wk = np.stack([L[i]["Wk"] for i in range(NL)]).astype(_bf)
    wv = np.stack([L[i]["Wv"] for i in range(NL)]).astype(_bf)
    wo = np.stack([L[i]["Wo"] for i in range(NL)]).astype(_bf)
    wl = np.stack([L[i]["Wl"] for i in range(NL)]).astype(_bf)
    wf1 = np.stack([L[i]["Wf1"] for i in range(NL)]).astype(_bf)
    wf2 = np.stack([L[i]["Wf2"] for i in range(NL)]).astype(_bf)

    shared = {
        "colb_idx": colb_idx,
        "has_gnb": has_gnb,
        "gm": gm,
        "bt": bt,
        "ncolb": colb.shape[0],
    }

    in_maps = []
    for core in range(NCORES):
        b, r = core // 2, core % 2
        lo, hi = r * VH, (r + 1) * VH
        cb = colb.copy()
        cb[colb_idx["gm0"] : colb_idx["gm0"] + 2] = gm[b].reshape(2, 128)
        cb[colb_idx["bt0"] : colb_idx["bt0"] + 2] = bt[b].reshape(2, 128)
        m = {
            "h0": np.concatenate([x[b], cond_x], axis=-1).astype(_bf),
            "mt": MT[:, lo:hi].astype(_bf),
            "xskip": x[b, lo:hi].astype(np.float32),
            "cond": condT[:, :, lo:hi].astype(np.float32),
            "wg0": wg0,
            "wg12": wg12,
            "wq": wq,
            "wk": wk,
            "wv": wv,
            "wo": wo,
            "wl": wl,
            "wf1": wf1,
            "wf2": wf2,
            "colb": cb,
            "natb": natb,
            "rowb": np.concatenate(
                [L[i]["bo"] for i in range(NL)] + [L[i]["bf2"] for i in range(NL)]
            ).reshape(1, -1).astype(_bf),
            "w0": w0.reshape(1, HID).astype(_bf),
            "msum": msum[lo:hi].reshape(1, VH).astype(_bf),
        }
        in_maps.append(m)
    return shared, in_maps


# ---------------------------------------------------------------------------
# Device program
# ---------------------------------------------------------------------------

_PROG_CACHE = {}


def _build_program(has_gnb, colb_idx, ncolb):
    import contextlib

    import concourse.tile as tile
    from concourse import mybir, bacc

    bf16 = mybir.dt.bfloat16
    f32 = mybir.dt.float32
    AF = mybir.ActivationFunctionType
    OP = mybir.AluOpType

    nc = bacc.Bacc("TRN2", target_bir_lowering=False, debug=False, num_devices=NCORES)

    def inp(name, shape, dt_):
        return nc.declare_dram_parameter(name, shape, dt_, isOutput=False)

    h0_d = inp("h0", [V, F0], bf16)
    mt_d = inp("mt", [V, VH], bf16)
    xskip_d = inp("xskip", [VH, HID], f32)
    cond_d = inp("cond", [NL, HID, VH], f32)
    wg0_d = inp("wg0", [F0, HID], bf16)
    wg12_d = inp("wg12", [2, HID, HID], bf16)
    wq_d = inp("wq", [NL, HID, HID], bf16)
    wk_d = inp("wk", [NL, HID, HID], bf16)
    wv_d = inp("wv", [NL, HID, HID], bf16)
    wo_d = inp("wo", [NL, HID, HID], bf16)
    wl_d = inp("wl", [NL, HID, HID], bf16)
    wf1_d = inp("wf1", [NL, HID, FF], bf16)
    wf2_d = inp("wf2", [NL, FF, HID], bf16)
    colb_d = inp("colb", [ncolb, 128], f32)
    natb_d = inp("natb", [2 * NL, 128, HID], f32)
    rowb_d = inp("rowb", [1, 12 * 128], bf16)
    w0_d = inp("w0", [1, HID], bf16)
    msum_d = inp("msum", [1, VH], bf16)
    out_d = nc.declare_dram_parameter("out", [VH, HID], f32, isOutput=True)

    kv_in = [nc.dram_tensor(f"kv_in{l}", [KVFLAT], bf16) for l in range(NL)]
    kv_out = [nc.dram_tensor(f"kv_out{l}", [2, KVFLAT], bf16) for l in range(NL)]
    hg_in = [None] + [nc.dram_tensor(f"hg_in{l}", [VH, HID], bf16) for l in (1, 2)]
    hg_out = [None] + [nc.dram_tensor(f"hg_out{l}", [2 * VH, HID], bf16) for l in (1, 2)]

    PAIRS = [[0, 1], [2, 3], [4, 5], [6, 7]]
    SCALE = float(1.0 / np.sqrt(DH))

    with tile.TileContext(nc) as tc, contextlib.ExitStack() as ctx:
        cp = ctx.enter_context(tc.tile_pool(name="const", bufs=1))
        wp = ctx.enter_context(tc.tile_pool(name="work", bufs=1))
        sp = ctx.enter_context(tc.tile_pool(name="small", bufs=1))
        rp = ctx.enter_context(tc.tile_pool(name="rows", bufs=2))
        esp = ctx.enter_context(tc.tile_pool(name="esp", bufs=3))
        pm = ctx.enter_context(tc.tile_pool(name="pm", bufs=2, space="PSUM"))
        psc = ctx.enter_context(tc.tile_pool(name="psc", bufs=2, space="PSUM"))
        pox = ctx.enter_context(tc.tile_pool(name="pox", bufs=1, space="PSUM"))

        # ---------------- persistent loads ----------------
        mt_sb = cp.tile([128, ST, VH], bf16)
        nc.sync.dma_start(mt_sb[:], mt_d.rearrange("(t p) d -> p t d", p=128))
        colb_sb = cp.tile([128, ncolb], f32)
        nc.sync.dma_start(colb_sb[:], colb_d.rearrange("n p -> p n"))
        natb_sb = cp.tile([128, 2 * NL, HID], f32)
        nc.sync.dma_start(natb_sb[:], natb_d.rearrange("n p f -> p n f"))

        wg0_sb = cp.tile([128, 3, HID], bf16)
        nc.vector.memset(wg0_sb[:], 0.0)
        nc.sync.dma_start(
            wg0_sb[:, 0:2, :], wg0_d[0:256].rearrange("(t p) f -> p t f", p=128)
        )
        nc.sync.dma_start(wg0_sb[:64, 2, :], wg0_d[256:320])

        rowb_sb = cp.tile([1, 12 * 128], bf16)
        nc.sync.dma_start(rowb_sb[:], rowb_d[:])
        ones_row = cp.tile([1, 512], bf16)
        nc.vector.memset(ones_row[:], 1.0)

        def rowb(name, j):
            l_ = int(name[-1])
            base = 0 if name.startswith("bo") else 6
            r = base + 2 * l_ + j
            return rowb_sb[0:1, r * 128 : (r + 1) * 128]

        w0_sb = cp.tile([1, HID], bf16)
        nc.sync.dma_start(w0_sb[:], w0_d[:])
        msum_sb = cp.tile([1, VH], bf16)
        nc.sync.dma_start(msum_sb[:], msum_d[:])

        ones_col = cp.tile([128, 1], bf16)
        nc.vector.memset(ones_col[:], 1.0)

        kTf_sb = cp.tile([128, 2, V], bf16)                 # gathered K^T
        vext_sb = cp.tile([128, ST, HEADS, DH + 1], bf16)   # gathered V nat + ones col
        nc.vector.memset(vext_sb[:], 1.0)

        def col(name, j):
            return colb_sb[:, colb_idx[name] + j : colb_idx[name] + j + 1]

        # h0 shares the full-V slot used by the gathered h of later layers
        hfull = cp.tile([128, ST, F0], bf16, tag="hfullg")
        nc.sync.dma_start(hfull[:], h0_d.rearrange("(t p) f -> p t f", p=128))
        hf_tiles = 3

        for l in range(NL):
            final = l == NL - 1

            # ---- per-layer weights ----
            wg_l = wg0_sb
            if l > 0:
                wg_l = cp.tile([128, 3, HID], bf16, tag="wg_l")
                nc.sync.dma_start(
                    wg_l[:, 0:2, :],
                    wg12_d[l - 1].rearrange("(t p) f -> p t f", p=128),
                )

            def loadw(d, kt, fo, tag):
                sb = cp.tile([128, kt, fo], bf16, tag=tag)
                nc.sync.dma_start(sb[:], d[l].rearrange("(t p) f -> p t f", p=128))
                return sb

            wq_sb = loadw(wq_d, 2, HID, "wq")
            wk_sb = loadw(wk_d, 2, HID, "wk")
            wv_sb = loadw(wv_d, 2, HID, "wv")
            wo_sb = loadw(wo_d, 2, HID, "wo")
            wl_sb = loadw(wl_d, 2, HID, "wl")
            wf1_sb = loadw(wf1_d, 2, FF, "wf1")
            wf2_sb = loadw(wf2_d, 8, HID, "wf2")
            cond_sb = cp.tile([128, 2, VH], f32, tag="cond_l")
            nc.sync.dma_start(
                cond_sb[:], cond_d[l].rearrange("(u p) v -> p u v", p=128)
            )

            # ---- gnorm (layer 1): scalar affine applied in place ----
            if l == 1:
                hsq = wp.tile([128, ST, HID], bf16, tag="hres")
                nc.vector.tensor_tensor(hsq[:], hfull[:, :, 0:HID], hfull[:, :, 0:HID], OP.mult)
                s_ps = pm.tile([128, 512], f32, tag="mm")
                q_ps = pm.tile([128, 512], f32, tag="mm")
                for st in range(ST):
                    nc.tensor.matmul(
                        s_ps[0:1, 0:HID], ones_col[:], hfull[:, st, 0:HID],
                        start=(st == 0), stop=(st == ST - 1),
                    )
                for st in range(ST):
                    nc.tensor.matmul(
                        q_ps[0:1, 0:HID], ones_col[:], hsq[:, st, :],
                        start=(st == 0), stop=(st == ST - 1),
                    )
                gn = rp.tile([1, 8], f32, tag="gn")
                nc.vector.reduce_sum(gn[:, 0:1], s_ps[0:1, 0:HID], mybir.AxisListType.X)
                nc.vector.reduce_sum(gn[:, 1:2], q_ps[0:1, 0:HID], mybir.AxisListType.X)
                VC = float(V * HID)
                nc.vector.tensor_scalar_mul(gn[:, 0:1], gn[:, 0:1], 1.0 / VC)  # mu
                nc.vector.tensor_scalar_mul(gn[:, 1:2], gn[:, 1:2], 1.0 / VC)  # E[x^2]
                nc.vector.tensor_tensor(gn[:, 2:3], gn[:, 0:1], gn[:, 0:1], OP.mult)
                nc.vector.tensor_tensor(gn[:, 2:3], gn[:, 1:2], gn[:, 2:3], OP.subtract)
                nc.vector.tensor_scalar_add(gn[:, 2:3], gn[:, 2:3], 1e-5)  # var+eps
                nc.scalar.activation(gn[:, 3:4], gn[:, 2:3], AF.Ln)
                nc.scalar.activation(gn[:, 4:5], gn[:, 3:4], AF.Exp, scale=-0.5)
                mu_c = sp.tile([128, 1], f32, tag="gmu")
                rs_c = sp.tile([128, 1], f32, tag="grs")
                nc.gpsimd.partition_broadcast(mu_c[:], gn[:, 0:1])
                nc.gpsimd.partition_broadcast(rs_c[:], gn[:, 4:5])
                nc.vector.tensor_scalar(
                    out=hfull[:, :, 0:HID], in0=hfull[:, :, 0:HID],
                    scalar1=mu_c[:], scalar2=rs_c[:],
                    op0=OP.subtract, op1=OP.mult,
                )

            # ---- gcn: zT = (M @ h)^T, then yT = (z @ Wg)^T ----
            zT = wp.tile([128, 3, VH], bf16, tag="zT")
            for ft in range(hf_tiles):
                fw = 64 if (l == 0 and ft == 2) else 128
                for qc in range(QC):
                    z_ps = pm.tile([128, 512], f32, tag="mm")
                    for st in range(ST):
                        nc.tensor.matmul(
                            z_ps[0:fw, :],
                            hfull[:, st, ft * 128 : ft * 128 + fw],
                            mt_sb[:, st, qc * 512 : (qc + 1) * 512],
                            start=(st == 0), stop=(st == ST - 1),
                        )
                    nc.vector.tensor_copy(
                        zT[0:fw, ft, qc * 512 : (qc + 1) * 512], z_ps[0:fw, :]
                    )

            gnb_term = has_gnb and l == 1
            hres = wp.tile([128, 2, VH], f32, tag="hres")
            hbf = wp.tile([128, 2, VH], bf16, tag="hbf")
            for u in range(2):
                for qc in range(QC):
                    y_ps = pm.tile([128, 512], f32, tag="mm")
                    for ft in range(hf_tiles):
                        fw = 64 if (l == 0 and ft == 2) else 128
                        nc.tensor.matmul(
                            y_ps[:],
                            wg_l[0:fw, ft, u * 128 : (u + 1) * 128],
                            zT[0:fw, ft, qc * 512 : (qc + 1) * 512],
                            start=(ft == 0),
                            stop=(ft == hf_tiles - 1 and not gnb_term),
                        )
                    if gnb_term:
                        nc.tensor.matmul(
                            y_ps[:],
                            w0_sb[:, u * 128 : (u + 1) * 128],
                            msum_sb[:, qc * 512 : (qc + 1) * 512],
                            start=False, stop=True,
                        )
                    qs = slice(qc * 512, (qc + 1) * 512)
                    if not final:
                        nc.vector.tensor_scalar(
                            out=hres[:, u, qs], in0=y_ps[:], scalar1=col(f"bg{l}", u),
                            scalar2=0.0, op0=OP.add, op1=OP.max,
                        )
                    else:
                        nc.vector.tensor_scalar(
                            out=hres[:, u, qs], in0=y_ps[:], scalar1=col(f"bg{l}", u),
                            scalar2=col("gm0", u), op0=OP.add, op1=OP.mult,
                        )
                        nc.vector.tensor_scalar(
                            out=hres[:, u, qs], in0=hres[:, u, qs],
                            scalar1=col("bt0", u), scalar2=0.0,
                            op0=OP.add, op1=OP.max,
                        )
                    nc.vector.tensor_tensor(
                        hres[:, u, qs], hres[:, u, qs], cond_sb[:, u, qs], OP.add
                    )
                    nc.vector.tensor_copy(hbf[:, u, qs], hres[:, u, qs])

            # ---- k, v projections + pair allgather ----
            kT_own = wp.tile([128, 2, VH], bf16, tag="kT_own")
            for u in range(2):
                for qc in range(QC):
                    k_ps = pm.tile([128, 512], f32, tag="mm")
                    for kt in range(2):
                        nc.tensor.matmul(
                            k_ps[:],
                            wk_sb[:, kt, u * 128 : (u + 1) * 128],
                            hbf[:, kt, qc * 512 : (qc + 1) * 512],
                            start=(kt == 0), stop=(kt == 1),
                        )
                    nc.vector.tensor_scalar(
                        out=kT_own[:, u, qc * 512 : (qc + 1) * 512], in0=k_ps[:],
                        scalar1=col(f"bk{l}", u), scalar2=None, op0=OP.add,
                    )
            vnat = wp.tile([128, VS, HID], bf16, tag="vnat")
            for vs in range(VS):
                v_ps = pm.tile([128, 512], f32, tag="mm")
                for kt in range(2):
                    nc.tensor.matmul(
                        v_ps[:, 0:HID],
                        hbf[:, kt, vs * 128 : (vs + 1) * 128],
                        wv_sb[:, kt, :],
                        start=(kt == 0), stop=(kt == 1),
                    )
                nc.vector.tensor_tensor(
                    vnat[:, vs, :], v_ps[:, 0:HID], natb_sb[:, l, :], OP.add
                )
            nc.sync.dma_start(
                kv_in[l][0 : HID * VH].rearrange("(p u v) -> p u v", p=128, u=2),
                kT_own[:],
            )
            nc.sync.dma_start(
                kv_in[l][HID * VH :].rearrange("(t p) f -> p t f", p=128)
                if False else
                kv_in[l][HID * VH :].rearrange("(t p f) -> p t f", p=128, f=HID),
                vnat[:],
            )
            nc.gpsimd.collective_compute(
                "AllGather", mybir.AluOpType.bypass,
                replica_groups=PAIRS,
                ins=[kv_in[l][:].opt()], outs=[kv_out[l][:].opt()],
            )
            for g in range(2):
                nc.gpsimd.dma_start(
                    kTf_sb[:, :, g * VH : (g + 1) * VH],
                    kv_out[l][g, 0 : HID * VH].rearrange(
                        "(p u v) -> p u v", p=128, u=2
                    ),
                )
                for st8 in range(8):
                    nc.gpsimd.dma_start(
                        vext_sb[:, 8 * g + st8, :, 0:DH],
                        kv_out[l][
                            g,
                            HID * VH + st8 * 128 * HID : HID * VH + (st8 + 1) * 128 * HID,
                        ].rearrange("(p h d) -> p h d", p=128, h=HEADS),
                    )

            # ---- q projection ----
            qT = wp.tile([128, 2, VH], bf16, tag="qT")
            for u in range(2):
                for qc in range(QC):
                    q_ps = pm.tile([128, 512], f32, tag="mm")
                    for kt in range(2):
                        nc.tensor.matmul(
                            q_ps[:],
                            wq_sb[:, kt, u * 128 : (u + 1) * 128],
                            hbf[:, kt, qc * 512 : (qc + 1) * 512],
                            start=(kt == 0), stop=(kt == 1),
                        )
                    nc.vector.tensor_scalar(
                        out=qT[:, u, qc * 512 : (qc + 1) * 512], in0=q_ps[:],
                        scalar1=col(f"bq{l}", u), scalar2=None, op0=OP.add,
                    )

            # ---- attention core ----
            oT = wp.tile([128, 2, VH], bf16, tag="oT")
            for qc in range(QC):
                qs = slice(qc * 512, (qc + 1) * 512)
                for hg in range(4):          # head pairs
                    u = hg // 2
                    hh0 = (hg % 2) * 2
                    o_ps = pox.tile([DH + 1, 2, 512], f32, tag="oext")
                    for st in range(ST):
                        s_ps = psc.tile([128, 2, 512], f32, tag="sc")
                        for j in range(2):
                            hh = hh0 + j
                            nc.tensor.matmul(
                                s_ps[:, j, :],
                                kTf_sb[32 * hh : 32 * hh + 32, u,
                                       st * 128 : (st + 1) * 128],
                                qT[32 * hh : 32 * hh + 32, u, qs],
                                start=True, stop=True,
                                tile_position=(32 * hh, 0),
                            )
                        es = esp.tile([128, 2, 512], bf16, tag="es")
                        nc.scalar.activation(es[:], s_ps[:], AF.Exp, scale=SCALE)
                        for j in range(2):
                            h_abs = u * 4 + hh0 + j
                            nc.tensor.matmul(
                                o_ps[:, j, :],
                                vext_sb[:, st, h_abs, :],
                                es[:, j, :],
                                start=(st == 0), stop=(st == ST - 1),
                            )
                    srow = rp.tile([1, 2, 512], f32, tag="avs")
                    nc.vector.tensor_copy(srow[:], o_ps[DH : DH + 1, :, :])
                    sbc = rp.tile([DH, 2, 512], f32, tag="avb")
                    nc.gpsimd.partition_broadcast(sbc[:], srow[:])
                    nc.vector.reciprocal_approx_fast(out=sbc[:], in_=sbc[:])
                    for j in range(2):
                        hh = hh0 + j
                        nc.vector.tensor_tensor(
                            oT[32 * hh : 32 * hh + 32, u, qs],
                            o_ps[0:DH, j, :], sbc[:, j, :], OP.mult,
                        )

            # ---- h = h + o @ Wo + bo ----
            for u in range(2):
                for qc in range(QC):
                    qs = slice(qc * 512, (qc + 1) * 512)
                    w_ps = pm.tile([128, 512], f32, tag="mm")
                    for kt in range(2):
                        nc.tensor.matmul(
                            w_ps[:],
                            wo_sb[:, kt, u * 128 : (u + 1) * 128],
                            oT[:, kt, qs],
                            start=(kt == 0), stop=False,
                        )
                    nc.tensor.matmul(
                        w_ps[:], rowb(f"bo{l}", u), ones_row[:],
                        start=False, stop=True,
                    )
                    nc.vector.tensor_tensor(
                        hres[:, u, qs], hres[:, u, qs], w_ps[:], OP.add
                    )
                    nc.vector.tensor_copy(hbf[:, u, qs], hres[:, u, qs])

            # ---- ff ----
            for qc in range(QC):
                qs = slice(qc * 512, (qc + 1) * 512)
                f1 = wp.tile([128, 8, 512], bf16, tag="f1")
                for m in range(8):
                    f_ps = pm.tile([128, 512], f32, tag="mm")
                    for kt in range(2):
                        nc.tensor.matmul(
                            f_ps[:],
                            wf1_sb[:, kt, m * 128 : (m + 1) * 128],
                            hbf[:, kt, qs],
                            start=(kt == 0), stop=(kt == 1),
                        )
                    nc.vector.tensor_scalar(
                        out=f1[:, m, :], in0=f_ps[:],
                        scalar1=col(f"bf1_{l}", m), scalar2=0.0,
                        op0=OP.add, op1=OP.max,
                    )
                for u in range(2):
                    g_ps = pm.tile([128, 512], f32, tag="mm")
                    for kt in range(8):
                        nc.tensor.matmul(
                            g_ps[:],
                            wf2_sb[:, kt, u * 128 : (u + 1) * 128],
                            f1[:, kt, :],
                            start=(kt == 0), stop=False,
                        )
                    nc.tensor.matmul(
                        g_ps[:], rowb(f"bf2_{l}", u), ones_row[:],
                        start=False, stop=True,
                    )
                    nc.vector.tensor_tensor(
                        hres[:, u, qs], hres[:, u, qs], g_ps[:], OP.add
                    )
                    nc.vector.tensor_copy(hbf[:, u, qs], hres[:, u, qs])

            # ---- layernorm over channels (whole layer) ----
            hsq2v = wp.tile([128, 2, VH], bf16, tag="hsq")
            nc.vector.tensor_tensor(hsq2v[:], hbf[:], hbf[:], OP.mult)
            s1 = sp.tile([1, VH], f32, tag="lns1")
            s2 = sp.tile([1, VH], f32, tag="lns2")
            for qc in range(QC):
                r_ps = pm.tile([128, 512], f32, tag="mm")
                for u in range(2):
                    nc.tensor.matmul(
                        r_ps[0:1, :], ones_col[:],
                        hbf[:, u, qc * 512 : (qc + 1) * 512],
                        start=(u == 0), stop=(u == 1),
                    )
                nc.vector.tensor_scalar_mul(
                    s1[:, qc * 512 : (qc + 1) * 512], r_ps[0:1, :], 1.0 / HID
                )
                r2_ps = pm.tile([128, 512], f32, tag="mm")
                for u in range(2):
                    nc.tensor.matmul(
                        r2_ps[0:1, :], ones_col[:],
                        hsq2v[:, u, qc * 512 : (qc + 1) * 512],
                        start=(u == 0), stop=(u == 1),
                    )
                nc.vector.tensor_scalar_mul(
                    s2[:, qc * 512 : (qc + 1) * 512], r2_ps[0:1, :], 1.0 / HID
                )
            var = sp.tile([1, VH], f32, tag="lnvar")
            nc.vector.tensor_tensor(var[:], s1[:], s1[:], OP.mult)
            nc.vector.tensor_tensor(var[:], s2[:], var[:], OP.subtract)
            nc.vector.tensor_scalar_add(var[:], var[:], 1e-5)
            lnv = sp.tile([1, VH], f32, tag="lnlog")
            nc.scalar.activation(lnv[:], var[:], AF.Ln)
            rstd = var
            nc.scalar.activation(rstd[:], lnv[:], AF.Exp, scale=-0.5)
            nb = s2
            nc.vector.tensor_tensor(nb[:], s1[:], rstd[:], OP.mult)
            nc.vector.tensor_scalar_mul(nb[:], nb[:], -1.0)
            a_bc = sp.tile([128, VH], f32, tag="lnabc")
            b_bc = sp.tile([128, VH], f32, tag="lnbbc")
            nc.gpsimd.partition_broadcast(a_bc[:], rstd[:])
            nc.gpsimd.partition_broadcast(b_bc[:], nb[:])
            hln = wp.tile([128, 2, VH], bf16, tag="hln")
            for u in range(2):
                nc.vector.tensor_tensor(
                    hres[:, u, :], hres[:, u, :], a_bc[:], OP.mult
                )
                nc.vector.tensor_tensor(
                    hres[:, u, :], hres[:, u, :], b_bc[:], OP.add
                )
                nc.vector.tensor_scalar(
                    out=hln[:, u, :], in0=hres[:, u, :], scalar1=col(f"lng{l}", u),
                    scalar2=col(f"lnb{l}", u), op0=OP.mult, op1=OP.add,
                )

            # ---- Wl linear (natural out) ----
            if not final:
                hnat_bf = wp.tile([128, VS, HID], bf16, tag="hnatb")
                for vs in range(VS):
                    n_ps = pm.tile([128, 512], f32, tag="mm")
                    for kt in range(2):
                        nc.tensor.matmul(
                            n_ps[:, 0:HID],
                            hln[:, kt, vs * 128 : (vs + 1) * 128],
                            wl_sb[:, kt, :],
                            start=(kt == 0), stop=(kt == 1),
                        )
                    tmp = sp.tile([128, HID], f32, tag="wltmp")
                    nc.vector.tensor_tensor(
                        tmp[:], n_ps[:, 0:HID], natb_sb[:, NL + l, :], OP.add
                    )
                    nc.vector.tensor_scalar(
                        out=hnat_bf[:, vs, :], in0=tmp[:], scalar1=0.0,
                        scalar2=None, op0=OP.max,
                    )
                nc.sync.dma_start(
                    hg_in[l + 1][:].rearrange("(t p) f -> p t f", p=128), hnat_bf[:]
                )
            else:
                for vs in range(VS):
                    n_ps = pm.tile([128, 512], f32, tag="mm")
                    for kt in range(2):
                        nc.tensor.matmul(
                            n_ps[:, 0:HID],
                            hln[:, kt, vs * 128 : (vs + 1) * 128],
                            wl_sb[:, kt, :],
                            start=(kt == 0), stop=(kt == 1),
                        )
                    xs = sp.tile([128, HID], f32, tag="xstmp")
                    nc.sync.dma_start(xs[:], xskip_d[vs * 128 : (vs + 1) * 128, :])
                    tmp = sp.tile([128, HID], f32, tag="wltmp")
                    nc.vector.tensor_tensor(
                        tmp[:], n_ps[:, 0:HID], natb_sb[:, NL + l, :], OP.add
                    )
                    out2 = sp.tile([128, HID], f32, tag="outtmp")
                    nc.vector.tensor_tensor(out2[:], tmp[:], xs[:], OP.add)
                    nc.sync.dma_start(
                        out_d[vs * 128 : (vs + 1) * 128, :], out2[:]
                    )

            if not final:
                nc.gpsimd.collective_compute(
                    "AllGather", mybir.AluOpType.bypass,
                    replica_groups=PAIRS,
                    ins=[hg_in[l + 1][:].opt()], outs=[hg_out[l + 1][:].opt()],
                )
                hfull = cp.tile([128, ST, F0], bf16, tag="hfullg")
                nc.gpsimd.dma_start(
                    hfull[:, :, 0:HID],
                    hg_out[l + 1].rearrange("(t p) f -> p t f", p=128),
                )
                hf_tiles = 2

    nc.finalize()
    return nc


LAST_EXEC_NS = None


def kernel(x, cond_x, edge_index, t, params):
    import os
    from concourse.bass_utils import run_bass_kernel_spmd

    shared, in_maps = _host_prep(x, cond_x, edge_index, t, params)

    key = (shared["has_gnb"],)
    if key not in _PROG_CACHE:
        _PROG_CACHE[key] = _build_program(
            shared["has_gnb"], shared["colb_idx"], shared["ncolb"]
        )
    nc = _PROG_CACHE[key]

    trace = os.environ.get("ATTGNN_TRACE", "0") == "1"
    r = run_bass_kernel_spmd(nc, in_maps, list(range(NCORES)), trace=trace)
    global LAST_EXEC_NS
    LAST_EXEC_NS = r.exec_time_ns
    out = np.zeros((B, V, HID), np.float32)
    for core in range(NCORES):
        b, rr = core // 2, core % 2
        out[b, rr * VH : (rr + 1) * VH] = r.results[core]["out"]
    return out


# revision 18
# speedup vs baseline: 1.1442x; 1.0233x over previous
"""Trainium2 Bass kernel for nn_AttGNN (3-layer GCN+attention network).

Sharding: 8 NeuronCores = 4 batch elements x 2-way node split (each core
owns 1024 of the 2048 nodes). Pair AllGathers supply the full node set
where needed (GCN aggregation input, attention K/V).

GCN message passing is reformulated as a dense matmul against the
host-precomputed symmetric-normalized adjacency (self-loop folded into
the diagonal), reordered as (M @ h) @ Wg so only the natural-layout h is
needed. The GroupNorm per-channel gain is folded into Wg on the host.
"""

import sys

sys.path.insert(0, "/opt/trn_rl_repo")

import numpy as np
import ml_dtypes

B, V, E = 4, 2048, 65536
F_IN, HID, COND, ENC, HEADS, NL = 256, 256, 64, 128, 8, 3
FF = 4 * HID
VH = V // 2          # nodes owned per core
DH = HID // HEADS    # 32
F0 = F_IN + COND     # 320, first-layer input width
NCORES = 8
KVFLAT = HID * VH + VH * HID   # flattened kT + v_nat halves, in elements
ST = V // 128        # 16 s-tiles over full V
VS = VH // 128       # 8 v-slices over own nodes
QC = VH // 512       # 2 q-chunks of 512

_bf = ml_dtypes.bfloat16


def _np(a):
    return np.asarray(a)


def _host_prep(x, cond_x, edge_index, t, params):
    """Build the per-core input maps (all numpy)."""
    x = _np(x).astype(np.float32)
    cond_x = _np(cond_x).astype(np.float32)
    ei = _np(edge_index).astype(np.int64)
    t = _np(t).astype(np.float32)

    src, dst = ei[0], ei[1]
    deg = np.zeros(V, np.float32)
    np.add.at(deg, dst, np.float32(1.0))
    deg += 1.0
    dinv = (1.0 / np.sqrt(deg)).astype(np.float32)

    # MT[s, d] = A[d, s]; A = normalized adjacency + diag(1/deg)
    MT = np.zeros((V, V), np.float32)
    np.add.at(MT, (src, dst), dinv[src] * dinv[dst])
    MT[np.arange(V), np.arange(V)] += 1.0 / deg
    msum = MT.sum(axis=0)  # row sums of A (for optional gn_b term)

    gn_g = _np(params["gn_g"]).astype(np.float32)
    gn_b = _np(params["gn_b"]).astype(np.float32)
    Wfilm = _np(params["Wfilm"]).astype(np.float32)
    bfilm = _np(params["bfilm"]).astype(np.float32)

    film = t @ Wfilm + bfilm          # [B, 2*HID]
    gm, bt = film[:, :HID], film[:, HID:]

    L = [
        {k: _np(v).astype(np.float32) for k, v in params["layers"][i].items()}
        for i in range(NL)
    ]

    # fold gn_g into Wg of layer 1 (the only gnorm'ed gcn input)
    Wg_eff = [L[0]["Wg"], gn_g[:, None] * L[1]["Wg"], L[2]["Wg"]]
    w0 = gn_b @ L[1]["Wg"]            # [HID]; nonzero only if gn_b != 0
    has_gnb = bool(np.any(gn_b != 0.0))

    condT = np.stack(
        [(cond_x @ L[i]["We"] + L[i]["be"]).T.astype(np.float32) for i in range(NL)]
    )  # [NL, HID, V]

    rows = []
    colb_idx = {}

    def add_col(name, vec):
        vec = vec.reshape(-1, 128)
        colb_idx[name] = len(rows)
        rows.extend(list(vec))

    for i in range(NL):
        add_col(f"bg{i}", L[i]["bg"])
        add_col(f"bq{i}", L[i]["bq"])
        add_col(f"bk{i}", L[i]["bk"])
        add_col(f"bo{i}", L[i]["bo"])
        add_col(f"bf2_{i}", L[i]["bf2"])
        add_col(f"lng{i}", L[i]["ln_g"])
        add_col(f"lnb{i}", L[i]["ln_b"])
        add_col(f"bf1_{i}", L[i]["bf1"])
    add_col("gm0", gm[0])   # per-core batch values patched in kernel()
    add_col("bt0", bt[0])
    colb = np.stack(rows).astype(np.float32)

    natb = np.stack(
        [np.broadcast_to(L[i]["bv"], (128, HID)) for i in range(NL)]
        + [np.broadcast_to(L[i]["bl"], (128, HID)) for i in range(NL)]
    ).astype(np.float32)

    wg0 = Wg_eff[0].astype(_bf)
    wg12 = np.stack([Wg_eff[1], Wg_eff[2]]).astype(_bf)
    wq = np.stack([L[i]["Wq"] for i in range(NL)]).astype(_bf)
    wk = np.stack([L[i]["Wk"] for i in range(NL)]).astype(_bf)
    wv = np.stack([L[i]["Wv"] for i in range(NL)]).astype(_bf)
    wo = np.stack([L[i]["Wo"] for i in range(NL)]).astype(_bf)
    wl = np.stack([L[i]["Wl"] for i in range(NL)]).astype(_bf)
    wf1 = np.stack([L[i]["Wf1"] for i in range(NL)]).astype(_bf)
    wf2 = np.stack([L[i]["Wf2"] for i in range(NL)]).astype(_bf)

    shared = {
        "colb_idx": colb_idx,
        "has_gnb": has_gnb,
        "gm": gm,
        "bt": bt,
        "ncolb": colb.shape[0],
    }

    in_maps = []
    for core in range(NCORES):
        b, r = core // 2, core % 2
        lo, hi = r * VH, (r + 1) * VH
        cb = colb.copy()
        cb[colb_idx["gm0"] : colb_idx["gm0"] + 2] = gm[b].reshape(2, 128)
        cb[colb_idx["bt0"] : colb_idx["bt0"] + 2] = bt[b].reshape(2, 128)
        m = {
            "h0": np.concatenate([x[b], cond_x], axis=-1).astype(_bf),
            "mt": MT[:, lo:hi].astype(_bf),
            "xskip": x[b, lo:hi].astype(np.float32),
            "cond": condT[:, :, lo:hi].astype(np.float32),
            "wg0": wg0,
            "wg12": wg12,
            "wq": wq,
            "wk": wk,
            "wv": wv,
            "wo": wo,
            "wl": wl,
            "wf1": wf1,
            "wf2": wf2,
            "colb": cb,
            "natb": natb,
            "rowb": np.concatenate(
                [L[i]["bo"] for i in range(NL)] + [L[i]["bf2"] for i in range(NL)]
            ).reshape(1, -1).astype(_bf),
            "w0": w0.reshape(1, HID).astype(_bf),
            "msum": msum[lo:hi].reshape(1, VH).astype(_bf),
        }
        in_maps.append(m)
    return shared, in_maps


# ---------------------------------------------------------------------------
# Device program
# ---------------------------------------------------------------------------

_PROG_CACHE = {}


def _build_program(has_gnb, colb_idx, ncolb):
    import contextlib

    import concourse.tile as tile
    from concourse import mybir, bacc

    bf16 = mybir.dt.bfloat16
    f32 = mybir.dt.float32
    AF = mybir.ActivationFunctionType
    OP = mybir.AluOpType

    nc = bacc.Bacc("TRN2", target_bir_lowering=False, debug=False, num_devices=NCORES)

    def inp(name, shape, dt_):
        return nc.declare_dram_parameter(name, shape, dt_, isOutput=False)

    h0_d = inp("h0", [V, F0], bf16)
    mt_d = inp("mt", [V, VH], bf16)
    xskip_d = inp("xskip", [VH, HID], f32)
    cond_d = inp("cond", [NL, HID, VH], f32)
    wg0_d = inp("wg0", [F0, HID], bf16)
    wg12_d = inp("wg12", [2, HID, HID], bf16)
    wq_d = inp("wq", [NL, HID, HID], bf16)
    wk_d = inp("wk", [NL, HID, HID], bf16)
    wv_d = inp("wv", [NL, HID, HID], bf16)
    wo_d = inp("wo", [NL, HID, HID], bf16)
    wl_d = inp("wl", [NL, HID, HID], bf16)
    wf1_d = inp("wf1", [NL, HID, FF], bf16)
    wf2_d = inp("wf2", [NL, FF, HID], bf16)
    colb_d = inp("colb", [ncolb, 128], f32)
    natb_d = inp("natb", [2 * NL, 128, HID], f32)
    rowb_d = inp("rowb", [1, 12 * 128], bf16)
    w0_d = inp("w0", [1, HID], bf16)
    msum_d = inp("msum", [1, VH], bf16)
    out_d = nc.declare_dram_parameter("out", [VH, HID], f32, isOutput=True)

    kv_in = [nc.dram_tensor(f"kv_in{l}", [KVFLAT], bf16) for l in range(NL)]
    kv_out = [nc.dram_tensor(f"kv_out{l}", [2, KVFLAT], bf16) for l in range(NL)]
    hg_in = [None] + [nc.dram_tensor(f"hg_in{l}", [VH, HID], bf16) for l in (1, 2)]
    hg_out = [None] + [nc.dram_tensor(f"hg_out{l}", [2 * VH, HID], bf16) for l in (1, 2)]

    PAIRS = [[0, 1], [2, 3], [4, 5], [6, 7]]
    SCALE = float(1.0 / np.sqrt(DH))

    with tile.TileContext(nc) as tc, contextlib.ExitStack() as ctx:
        cp = ctx.enter_context(tc.tile_pool(name="const", bufs=1))
        wp = ctx.enter_context(tc.tile_pool(name="work", bufs=1))
        sp = ctx.enter_context(tc.tile_pool(name="small", bufs=1))
        rp = ctx.enter_context(tc.tile_pool(name="rows", bufs=2))
        esp = ctx.enter_context(tc.tile_pool(name="esp", bufs=4))
        pm = ctx.enter_context(tc.tile_pool(name="pm", bufs=2, space="PSUM"))
        psc = ctx.enter_context(tc.tile_pool(name="psc", bufs=2, space="PSUM"))
        pox = ctx.enter_context(tc.tile_pool(name="pox", bufs=1, space="PSUM"))

        # ---------------- persistent loads ----------------
        mt_sb = cp.tile([128, ST, VH], bf16)
        nc.sync.dma_start(mt_sb[:], mt_d.rearrange("(t p) d -> p t d", p=128))
        colb_sb = cp.tile([128, ncolb], f32)
        nc.sync.dma_start(colb_sb[:], colb_d.rearrange("n p -> p n"))
        natb_sb = cp.tile([128, 2 * NL, HID], f32)
        nc.sync.dma_start(natb_sb[:], natb_d.rearrange("n p f -> p n f"))

        wg0_sb = cp.tile([128, 3, HID], bf16)
        nc.vector.memset(wg0_sb[:], 0.0)
        nc.sync.dma_start(
            wg0_sb[:, 0:2, :], wg0_d[0:256].rearrange("(t p) f -> p t f", p=128)
        )
        nc.sync.dma_start(wg0_sb[:64, 2, :], wg0_d[256:320])

        rowb_sb = cp.tile([1, 12 * 128], bf16)
        nc.sync.dma_start(rowb_sb[:], rowb_d[:])
        ones_row = cp.tile([1, 512], bf16)
        nc.vector.memset(ones_row[:], 1.0)

        def rowb(name, j):
            l_ = int(name[-1])
            base = 0 if name.startswith("bo") else 6
            r = base + 2 * l_ + j
            return rowb_sb[0:1, r * 128 : (r + 1) * 128]

        w0_sb = cp.tile([1, HID], bf16)
        nc.sync.dma_start(w0_sb[:], w0_d[:])
        msum_sb = cp.tile([1, VH], bf16)
        nc.sync.dma_start(msum_sb[:], msum_d[:])

        ones_col = cp.tile([128, 1], bf16)
        nc.vector.memset(ones_col[:], 1.0)

        kTf_sb = cp.tile([128, 2, V], bf16)                 # gathered K^T
        vext_sb = cp.tile([128, ST, HEADS, DH + 1], bf16)   # gathered V nat + ones col
        nc.vector.memset(vext_sb[:], 1.0)

        def col(name, j):
            return colb_sb[:, colb_idx[name] + j : colb_idx[name] + j + 1]

        # h0 shares the full-V slot used by the gathered h of later layers
        hfull = cp.tile([128, ST, F0], bf16, tag="hfullg")
        nc.sync.dma_start(hfull[:], h0_d.rearrange("(t p) f -> p t f", p=128))
        hf_tiles = 3

        for l in range(NL):
            final = l == NL - 1

            # ---- per-layer weights ----
            wg_l = wg0_sb
            if l > 0:
                wg_l = cp.tile([128, 3, HID], bf16, tag="wg_l")
                nc.sync.dma_start(
                    wg_l[:, 0:2, :],
                    wg12_d[l - 1].rearrange("(t p) f -> p t f", p=128),
                )

            def loadw(d, kt, fo, tag):
                sb = cp.tile([128, kt, fo], bf16, tag=tag)
                nc.sync.dma_start(sb[:], d[l].rearrange("(t p) f -> p t f", p=128))
                return sb

            wq_sb = loadw(wq_d, 2, HID, "wq")
            wk_sb = loadw(wk_d, 2, HID, "wk")
            wv_sb = loadw(wv_d, 2, HID, "wv")
            wo_sb = loadw(wo_d, 2, HID, "wo")
            wl_sb = loadw(wl_d, 2, HID, "wl")
            wf1_sb = loadw(wf1_d, 2, FF, "wf1")
            wf2_sb = loadw(wf2_d, 8, HID, "wf2")
            cond_sb = cp.tile([128, 2, VH], f32, tag="cond_l")
            nc.sync.dma_start(
                cond_sb[:], cond_d[l].rearrange("(u p) v -> p u v", p=128)
            )

            # ---- gnorm (layer 1): scalar affine applied in place ----
            if l == 1:
                hsq = wp.tile([128, ST, HID], bf16, tag="hres")
                nc.vector.tensor_tensor(hsq[:], hfull[:, :, 0:HID], hfull[:, :, 0:HID], OP.mult)
                s_ps = pm.tile([128, 512], f32, tag="mm")
                q_ps = pm.tile([128, 512], f32, tag="mm")
                for st in range(ST):
                    nc.tensor.matmul(
                        s_ps[0:1, 0:HID], ones_col[:], hfull[:, st, 0:HID],
                        start=(st == 0), stop=(st == ST - 1),
                    )
                for st in range(ST):
                    nc.tensor.matmul(
                        q_ps[0:1, 0:HID], ones_col[:], hsq[:, st, :],
                        start=(st == 0), stop=(st == ST - 1),
                    )
                gn = rp.tile([1, 8], f32, tag="gn")
                nc.vector.reduce_sum(gn[:, 0:1], s_ps[0:1, 0:HID], mybir.AxisListType.X)
                nc.vector.reduce_sum(gn[:, 1:2], q_ps[0:1, 0:HID], mybir.AxisListType.X)
                VC = float(V * HID)
                nc.vector.tensor_scalar_mul(gn[:, 0:1], gn[:, 0:1], 1.0 / VC)  # mu
                nc.vector.tensor_scalar_mul(gn[:, 1:2], gn[:, 1:2], 1.0 / VC)  # E[x^2]
                nc.vector.tensor_tensor(gn[:, 2:3], gn[:, 0:1], gn[:, 0:1], OP.mult)
                nc.vector.tensor_tensor(gn[:, 2:3], gn[:, 1:2], gn[:, 2:3], OP.subtract)
                nc.vector.tensor_scalar_add(gn[:, 2:3], gn[:, 2:3], 1e-5)  # var+eps
                nc.scalar.activation(gn[:, 3:4], gn[:, 2:3], AF.Ln)
                nc.scalar.activation(gn[:, 4:5], gn[:, 3:4], AF.Exp, scale=-0.5)
                mu_c = sp.tile([128, 1], f32, tag="gmu")
                rs_c = sp.tile([128, 1], f32, tag="grs")
                nc.gpsimd.partition_broadcast(mu_c[:], gn[:, 0:1])
                nc.gpsimd.partition_broadcast(rs_c[:], gn[:, 4:5])
                nc.vector.tensor_scalar(
                    out=hfull[:, :, 0:HID], in0=hfull[:, :, 0:HID],
                    scalar1=mu_c[:], scalar2=rs_c[:],
                    op0=OP.subtract, op1=OP.mult,
                )

            # ---- gcn: zT = (M @ h)^T, then yT = (z @ Wg)^T ----
            zT = wp.tile([128, 3, VH], bf16, tag="zT")
            for ft in range(hf_tiles):
                fw = 64 if (l == 0 and ft == 2) else 128
                for qc in range(QC):
                    z_ps = pm.tile([128, 512], f32, tag="mm")
                    for st in range(ST):
                        nc.tensor.matmul(
                            z_ps[0:fw, :],
                            hfull[:, st, ft * 128 : ft * 128 + fw],
                            mt_sb[:, st, qc * 512 : (qc + 1) * 512],
                            start=(st == 0), stop=(st == ST - 1),
                        )
                    nc.scalar.activation(
                        zT[0:fw, ft, qc * 512 : (qc + 1) * 512], z_ps[0:fw, :],
                        AF.Copy,
                    )

            gnb_term = has_gnb and l == 1
            hres = wp.tile([128, 2, VH], f32, tag="hres")
            hbf = wp.tile([128, 2, VH], bf16, tag="hbf")
            for u in range(2):
                for qc in range(QC):
                    y_ps = pm.tile([128, 512], f32, tag="mm")
                    for ft in range(hf_tiles):
                        fw = 64 if (l == 0 and ft == 2) else 128
                        nc.tensor.matmul(
                            y_ps[:],
                            wg_l[0:fw, ft, u * 128 : (u + 1) * 128],
                            zT[0:fw, ft, qc * 512 : (qc + 1) * 512],
                            start=(ft == 0),
                            stop=(ft == hf_tiles - 1 and not gnb_term),
                        )
                    if gnb_term:
                        nc.tensor.matmul(
                            y_ps[:],
                            w0_sb[:, u * 128 : (u + 1) * 128],
                            msum_sb[:, qc * 512 : (qc + 1) * 512],
                            start=False, stop=True,
                        )
                    qs = slice(qc * 512, (qc + 1) * 512)
                    if not final:
                        nc.vector.tensor_scalar(
                            out=hres[:, u, qs], in0=y_ps[:], scalar1=col(f"bg{l}", u),
                            scalar2=0.0, op0=OP.add, op1=OP.max,
                        )
                    else:
                        nc.vector.tensor_scalar(
                            out=hres[:, u, qs], in0=y_ps[:], scalar1=col(f"bg{l}", u),
                            scalar2=col("gm0", u), op0=OP.add, op1=OP.mult,
                        )
                        nc.vector.tensor_scalar(
                            out=hres[:, u, qs], in0=hres[:, u, qs],
                            scalar1=col("bt0", u), scalar2=0.0,
                            op0=OP.add, op1=OP.max,
                        )
                    nc.vector.tensor_tensor(
                        hres[:, u, qs], hres[:, u, qs], cond_sb[:, u, qs], OP.add
                    )
                    nc.vector.tensor_copy(hbf[:, u, qs], hres[:, u, qs])

            # ---- k, v projections + pair allgather ----
            kT_own = wp.tile([128, 2, VH], bf16, tag="kT_own")
            for u in range(2):
                for qc in range(QC):
                    k_ps = pm.tile([128, 512], f32, tag="mm")
                    for kt in range(2):
                        nc.tensor.matmul(
                            k_ps[:],
                            wk_sb[:, kt, u * 128 : (u + 1) * 128],
                            hbf[:, kt, qc * 512 : (qc + 1) * 512],
                            start=(kt == 0), stop=(kt == 1),
                        )
                    nc.vector.tensor_scalar(
                        out=kT_own[:, u, qc * 512 : (qc + 1) * 512], in0=k_ps[:],
                        scalar1=col(f"bk{l}", u), scalar2=None, op0=OP.add,
                    )
            vnat = wp.tile([128, VS, HID], bf16, tag="vnat")
            for vs in range(VS):
                v_ps = pm.tile([128, 512], f32, tag="mm")
                for kt in range(2):
                    nc.tensor.matmul(
                        v_ps[:, 0:HID],
                        hbf[:, kt, vs * 128 : (vs + 1) * 128],
                        wv_sb[:, kt, :],
                        start=(kt == 0), stop=(kt == 1),
                    )
                nc.vector.tensor_tensor(
                    vnat[:, vs, :], v_ps[:, 0:HID], natb_sb[:, l, :], OP.add
                )
            nc.sync.dma_start(
                kv_in[l][0 : HID * VH].rearrange("(p u v) -> p u v", p=128, u=2),
                kT_own[:],
            )
            nc.sync.dma_start(
                kv_in[l][HID * VH :].rearrange("(t p) f -> p t f", p=128)
                if False else
                kv_in[l][HID * VH :].rearrange("(t p f) -> p t f", p=128, f=HID),
                vnat[:],
            )
            nc.gpsimd.collective_compute(
                "AllGather", mybir.AluOpType.bypass,
                replica_groups=PAIRS,
                ins=[kv_in[l][:].opt()], outs=[kv_out[l][:].opt()],
            )
            for g in range(2):
                nc.gpsimd.dma_start(
                    kTf_sb[:, :, g * VH : (g + 1) * VH],
                    kv_out[l][g, 0 : HID * VH].rearrange(
                        "(p u v) -> p u v", p=128, u=2
                    ),
                )
                for st8 in range(8):
                    nc.gpsimd.dma_start(
                        vext_sb[:, 8 * g + st8, :, 0:DH],
                        kv_out[l][
                            g,
                            HID * VH + st8 * 128 * HID : HID * VH + (st8 + 1) * 128 * HID,
                        ].rearrange("(p h d) -> p h d", p=128, h=HEADS),
                    )

            # ---- q projection ----
            qT = wp.tile([128, 2, VH], bf16, tag="qT")
            for u in range(2):
                for qc in range(QC):
                    q_ps = pm.tile([128, 512], f32, tag="mm")
                    for kt in range(2):
                        nc.tensor.matmul(
                            q_ps[:],
                            wq_sb[:, kt, u * 128 : (u + 1) * 128],
                            hbf[:, kt, qc * 512 : (qc + 1) * 512],
                            start=(kt == 0), stop=(kt == 1),
                        )
                    nc.vector.tensor_scalar(
                        out=qT[:, u, qc * 512 : (qc + 1) * 512], in0=q_ps[:],
                        scalar1=col(f"bq{l}", u), scalar2=None, op0=OP.add,
                    )

            # ---- attention core ----
            oT = wp.tile([128, 2, VH], bf16, tag="oT")
            for qc in range(QC):
                qs = slice(qc * 512, (qc + 1) * 512)
                for hg in range(4):          # head pairs
                    u = hg // 2
                    hh0 = (hg % 2) * 2
                    o_ps = pox.tile([DH + 1, 2, 512], f32, tag="oext")
                    for st in range(ST):
                        s_ps = psc.tile([128, 2, 512], f32, tag="sc")
                        for j in range(2):
                            hh = hh0 + j
                            nc.tensor.matmul(
                                s_ps[:, j, :],
                                kTf_sb[32 * hh : 32 * hh + 32, u,
                                       st * 128 : (st + 1) * 128],
                                qT[32 * hh : 32 * hh + 32, u, qs],
                                start=True, stop=True,
                                tile_position=(32 * hh, 0),
                            )
                        es = esp.tile([128, 2, 512], bf16, tag="es")
                        nc.scalar.activation(es[:], s_ps[:], AF.Exp, scale=SCALE)
                        for j in range(2):
                            h_abs = u * 4 + hh0 + j
                            nc.tensor.matmul(
                                o_ps[:, j, :],
                                vext_sb[:, st, h_abs, :],
                                es[:, j, :],
                                start=(st == 0), stop=(st == ST - 1),
                            )
                    srow = rp.tile([1, 2, 512], f32, tag="avs")
                    nc.vector.tensor_copy(srow[:], o_ps[DH : DH + 1, :, :])
                    sbc = rp.tile([DH, 2, 512], f32, tag="avb")
                    nc.gpsimd.partition_broadcast(sbc[:], srow[:])
                    nc.vector.reciprocal_approx_fast(out=sbc[:], in_=sbc[:])
                    for j in range(2):
                        hh = hh0 + j
                        nc.vector.tensor_tensor(
                            oT[32 * hh : 32 * hh + 32, u, qs],
                            o_ps[0:DH, j, :], sbc[:, j, :], OP.mult,
                        )
            # ---- h = h + o @ Wo + bo ----
            for u in range(2):
                for qc in range(QC):
                    qs = slice(qc * 512, (qc + 1) * 512)
                    w_ps = pm.tile([128, 512], f32, tag="mm")
                    for kt in range(2):
                        nc.tensor.matmul(
                            w_ps[:],
                            wo_sb[:, kt, u * 128 : (u + 1) * 128],
                            oT[:, kt, qs],
                            start=(kt == 0), stop=False,
                        )
                    nc.tensor.matmul(
                        w_ps[:], rowb(f"bo{l}", u), ones_row[:],
                        start=False, stop=True,
                    )
                    nc.vector.tensor_tensor(
                        hres[:, u, qs], hres[:, u, qs], w_ps[:], OP.add
                    )
                    nc.vector.tensor_copy(hbf[:, u, qs], hres[:, u, qs])

            # ---- ff ----
            for qc in range(QC):
                qs = slice(qc * 512, (qc + 1) * 512)
                f1 = wp.tile([128, 8, 512], bf16, tag="f1")
                for m in range(8):
                    f_ps = pm.tile([128, 512], f32, tag="mm")
                    for kt in range(2):
                        nc.tensor.matmul(
                            f_ps[:],
                            wf1_sb[:, kt, m * 128 : (m + 1) * 128],
                            hbf[:, kt, qs],
                            start=(kt == 0), stop=(kt == 1),
                        )
                    nc.scalar.activation(
                        f1[:, m, :], f_ps[:], AF.Relu, bias=col(f"bf1_{l}", m),
                    )
                for u in range(2):
                    g_ps = pm.tile([128, 512], f32, tag="mm")
                    for kt in range(8):
                        nc.tensor.matmul(
                            g_ps[:],
                            wf2_sb[:, kt, u * 128 : (u + 1) * 128],
                            f1[:, kt, :],
                            start=(kt == 0), stop=False,
                        )
                    nc.tensor.matmul(
                        g_ps[:], rowb(f"bf2_{l}", u), ones_row[:],
                        start=False, stop=True,
                    )
                    nc.vector.tensor_tensor(
                        hres[:, u, qs], hres[:, u, qs], g_ps[:], OP.add
                    )
                    nc.vector.tensor_copy(hbf[:, u, qs], hres[:, u, qs])

            # ---- layernorm over channels (whole layer) ----
            hsq2v = wp.tile([128, 2, VH], bf16, tag="hsq")
            nc.vector.tensor_tensor(hsq2v[:], hbf[:], hbf[:], OP.mult)
            s1 = sp.tile([1, VH], f32, tag="lns1")
            s2 = sp.tile([1, VH], f32, tag="lns2")
            for qc in range(QC):
                r_ps = pm.tile([128, 512], f32, tag="mm")
                for u in range(2):
                    nc.tensor.matmul(
                        r_ps[0:1, :], ones_col[:],
                        hbf[:, u, qc * 512 : (qc + 1) * 512],
                        start=(u == 0), stop=(u == 1),
                    )
                nc.vector.tensor_scalar_mul(
                    s1[:, qc * 512 : (qc + 1) * 512], r_ps[0:1, :], 1.0 / HID
                )
                r2_ps = pm.tile([128, 512], f32, tag="mm")
                for u in range(2):
                    nc.tensor.matmul(
                        r2_ps[0:1, :], ones_col[:],
                        hsq2v[:, u, qc * 512 : (qc + 1) * 512],
                        start=(u == 0), stop=(u == 1),
                    )
                nc.vector.tensor_scalar_mul(
                    s2[:, qc * 512 : (qc + 1) * 512], r2_ps[0:1, :], 1.0 / HID
                )
            var = sp.tile([1, VH], f32, tag="lnvar")
            nc.vector.tensor_tensor(var[:], s1[:], s1[:], OP.mult)
            nc.vector.tensor_tensor(var[:], s2[:], var[:], OP.subtract)
            nc.vector.tensor_scalar_add(var[:], var[:], 1e-5)
            lnv = sp.tile([1, VH], f32, tag="lnlog")
            nc.scalar.activation(lnv[:], var[:], AF.Ln)
            rstd = var
            nc.scalar.activation(rstd[:], lnv[:], AF.Exp, scale=-0.5)
            nb = s2
            nc.vector.tensor_tensor(nb[:], s1[:], rstd[:], OP.mult)
            nc.vector.tensor_scalar_mul(nb[:], nb[:], -1.0)
            a_bc = sp.tile([128, VH], f32, tag="lnabc")
            b_bc = sp.tile([128, VH], f32, tag="lnbbc")
            nc.gpsimd.partition_broadcast(a_bc[:], rstd[:])
            nc.gpsimd.partition_broadcast(b_bc[:], nb[:])
            hln = wp.tile([128, 2, VH], bf16, tag="hln")
            for u in range(2):
                nc.vector.tensor_tensor(
                    hres[:, u, :], hres[:, u, :], a_bc[:], OP.mult
                )
                nc.vector.tensor_tensor(
                    hres[:, u, :], hres[:, u, :], b_bc[:], OP.add
                )
                nc.vector.tensor_scalar(
                    out=hln[:, u, :], in0=hres[:, u, :], scalar1=col(f"lng{l}", u),
                    scalar2=col(f"lnb{l}", u), op0=OP.mult, op1=OP.add,
                )

            # ---- Wl linear (natural out) ----
            if not final:
                hnat_bf = wp.tile([128, VS, HID], bf16, tag="hnatb")
                for vs in range(VS):
                    n_ps = pm.tile([128, 512], f32, tag="mm")
                    for kt in range(2):
                        nc.tensor.matmul(
                            n_ps[:, 0:HID],
                            hln[:, kt, vs * 128 : (vs + 1) * 128],
                            wl_sb[:, kt, :],
                            start=(kt == 0), stop=(kt == 1),
                        )
                    tmp = sp.tile([128, HID], f32, tag="wltmp")
                    nc.vector.tensor_tensor(
                        tmp[:], n_ps[:, 0:HID], natb_sb[:, NL + l, :], OP.add
                    )
                    nc.vector.tensor_scalar(
                        out=hnat_bf[:, vs, :], in0=tmp[:], scalar1=0.0,
                        scalar2=None, op0=OP.max,
                    )
                nc.sync.dma_start(
                    hg_in[l + 1][:].rearrange("(t p) f -> p t f", p=128), hnat_bf[:]
                )
            else:
                for vs in range(VS):
                    n_ps = pm.tile([128, 512], f32, tag="mm")
                    for kt in range(2):
                        nc.tensor.matmul(
                            n_ps[:, 0:HID],
                            hln[:, kt, vs * 128 : (vs + 1) * 128],
                            wl_sb[:, kt, :],
                            start=(kt == 0), stop=(kt == 1),
                        )
                    xs = sp.tile([128, HID], f32, tag="xstmp")
                    nc.sync.dma_start(xs[:], xskip_d[vs * 128 : (vs + 1) * 128, :])
                    tmp = sp.tile([128, HID], f32, tag="wltmp")
                    nc.vector.tensor_tensor(
                        tmp[:], n_ps[:, 0:HID], natb_sb[:, NL + l, :], OP.add
                    )
                    out2 = sp.tile([128, HID], f32, tag="outtmp")
                    nc.vector.tensor_tensor(out2[:], tmp[:], xs[:], OP.add)
                    nc.sync.dma_start(
                        out_d[vs * 128 : (vs + 1) * 128, :], out2[:]
                    )

            if not final:
                nc.gpsimd.collective_compute(
                    "AllGather", mybir.AluOpType.bypass,
                    replica_groups=PAIRS,
                    ins=[hg_in[l + 1][:].opt()], outs=[hg_out[l + 1][:].opt()],
                )
                hfull = cp.tile([128, ST, F0], bf16, tag="hfullg")
                nc.gpsimd.dma_start(
                    hfull[:, :, 0:HID],
                    hg_out[l + 1].rearrange("(t p) f -> p t f", p=128),
                )
                hf_tiles = 2

    nc.finalize()
    return nc


LAST_EXEC_NS = None


def kernel(x, cond_x, edge_index, t, params):
    import os
    from concourse.bass_utils import run_bass_kernel_spmd

    shared, in_maps = _host_prep(x, cond_x, edge_index, t, params)

    key = (shared["has_gnb"],)
    if key not in _PROG_CACHE:
        _PROG_CACHE[key] = _build_program(
            shared["has_gnb"], shared["colb_idx"], shared["ncolb"]
        )
    nc = _PROG_CACHE[key]

    trace = os.environ.get("ATTGNN_TRACE", "0") == "1"
    r = run_bass_kernel_spmd(nc, in_maps, list(range(NCORES)), trace=trace)
    global LAST_EXEC_NS
    LAST_EXEC_NS = r.exec_time_ns
    out = np.zeros((B, V, HID), np.float32)
    for core in range(NCORES):
        b, rr = core // 2, core % 2
        out[b, rr * VH : (rr + 1) * VH] = r.results[core]["out"]
    return out


# revision 19
# speedup vs baseline: 1.3192x; 1.1529x over previous
"""Trainium2 Bass kernel for nn_AttGNN (3-layer GCN+attention network).

Sharding: 8 NeuronCores = 4 batch elements x 2-way node split (each core
owns 1024 of the 2048 nodes). Pair AllGathers supply the full node set
where needed (GCN aggregation input, attention K/V).

GCN message passing is reformulated as a dense matmul against the
host-precomputed symmetric-normalized adjacency (self-loop folded into
the diagonal), reordered as (M @ h) @ Wg so only the natural-layout h is
needed. The GroupNorm per-channel gain is folded into Wg on the host.
"""

import sys

sys.path.insert(0, "/opt/trn_rl_repo")

import numpy as np
import ml_dtypes

B, V, E = 4, 2048, 65536
F_IN, HID, COND, ENC, HEADS, NL = 256, 256, 64, 128, 8, 3
FF = 4 * HID
VH = V // 2          # nodes owned per core
DH = HID // HEADS    # 32
F0 = F_IN + COND     # 320, first-layer input width
NCORES = 8
KVFLAT = HID * VH + VH * HID   # flattened kT + v_nat halves, in elements
ST = V // 128        # 16 s-tiles over full V
VS = VH // 128       # 8 v-slices over own nodes
QC = VH // 512       # 2 q-chunks of 512

_bf = ml_dtypes.bfloat16


def _np(a):
    return np.asarray(a)


def _host_prep(x, cond_x, edge_index, t, params):
    """Build the per-core input maps (all numpy)."""
    x = _np(x).astype(np.float32)
    cond_x = _np(cond_x).astype(np.float32)
    ei = _np(edge_index).astype(np.int64)
    t = _np(t).astype(np.float32)

    src, dst = ei[0], ei[1]
    deg = np.zeros(V, np.float32)
    np.add.at(deg, dst, np.float32(1.0))
    deg += 1.0
    dinv = (1.0 / np.sqrt(deg)).astype(np.float32)

    # MT[s, d] = A[d, s]; A = normalized adjacency + diag(1/deg)
    MT = np.zeros((V, V), np.float32)
    np.add.at(MT, (src, dst), dinv[src] * dinv[dst])
    MT[np.arange(V), np.arange(V)] += 1.0 / deg
    msum = MT.sum(axis=0)  # row sums of A (for optional gn_b term)

    gn_g = _np(params["gn_g"]).astype(np.float32)
    gn_b = _np(params["gn_b"]).astype(np.float32)
    Wfilm = _np(params["Wfilm"]).astype(np.float32)
    bfilm = _np(params["bfilm"]).astype(np.float32)

    film = t @ Wfilm + bfilm          # [B, 2*HID]
    gm, bt = film[:, :HID], film[:, HID:]

    L = [
        {k: _np(v).astype(np.float32) for k, v in params["layers"][i].items()}
        for i in range(NL)
    ]

    # fold gn_g into Wg of layer 1 (the only gnorm'ed gcn input)
    Wg_eff = [L[0]["Wg"], gn_g[:, None] * L[1]["Wg"], L[2]["Wg"]]
    w0 = gn_b @ L[1]["Wg"]            # [HID]; nonzero only if gn_b != 0
    has_gnb = bool(np.any(gn_b != 0.0))

    condT = np.stack(
        [(cond_x @ L[i]["We"] + L[i]["be"]).T.astype(np.float32) for i in range(NL)]
    )  # [NL, HID, V]

    rows = []
    colb_idx = {}

    def add_col(name, vec):
        vec = vec.reshape(-1, 128)
        colb_idx[name] = len(rows)
        rows.extend(list(vec))

    for i in range(NL):
        add_col(f"bg{i}", L[i]["bg"])
        add_col(f"bq{i}", L[i]["bq"])
        add_col(f"bk{i}", L[i]["bk"])
        add_col(f"bo{i}", L[i]["bo"])
        add_col(f"bf2_{i}", L[i]["bf2"])
        add_col(f"lng{i}", L[i]["ln_g"])
        add_col(f"lnb{i}", L[i]["ln_b"])
        add_col(f"bf1_{i}", L[i]["bf1"])
    add_col("gm0", gm[0])   # per-core batch values patched in kernel()
    add_col("bt0", bt[0])
    colb = np.stack(rows).astype(np.float32)

    natb = np.stack(
        [np.broadcast_to(L[i]["bv"], (128, HID)) for i in range(NL)]
        + [np.broadcast_to(L[i]["bl"], (128, HID)) for i in range(NL)]
    ).astype(np.float32)

    wg0 = Wg_eff[0].astype(_bf)
    wg12 = np.stack([Wg_eff[1], Wg_eff[2]]).astype(_bf)
    wq = np.stack([L[i]["Wq"] for i in range(NL)]).astype(_bf)
    wk = np.stack([L[i]["Wk"] for i in range(NL)]).astype(_bf)
    wv = np.stack([L[i]["Wv"] for i in range(NL)]).astype(_bf)
    wo = np.stack([L[i]["Wo"] for i in range(NL)]).astype(_bf)
    wl = np.stack([L[i]["Wl"] for i in range(NL)]).astype(_bf)
    wf1 = np.stack([L[i]["Wf1"] for i in range(NL)]).astype(_bf)
    wf2 = np.stack([L[i]["Wf2"] for i in range(NL)]).astype(_bf)

    shared = {
        "colb_idx": colb_idx,
        "has_gnb": has_gnb,
        "gm": gm,
        "bt": bt,
        "ncolb": colb.shape[0],
    }

    in_maps = []
    for core in range(NCORES):
        b, r = core // 2, core % 2
        lo, hi = r * VH, (r + 1) * VH
        cb = colb.copy()
        cb[colb_idx["gm0"] : colb_idx["gm0"] + 2] = gm[b].reshape(2, 128)
        cb[colb_idx["bt0"] : colb_idx["bt0"] + 2] = bt[b].reshape(2, 128)
        m = {
            "h0": np.concatenate([x[b], cond_x], axis=-1).astype(_bf),
            "mt": MT[:, lo:hi].astype(_bf),
            "xskip": x[b, lo:hi].astype(np.float32),
            "cond": condT[:, :, lo:hi].astype(np.float32),
            "wg0": wg0,
            "wg12": wg12,
            "wq": wq,
            "wk": wk,
            "wv": wv,
            "wo": wo,
            "wl": wl,
            "wf1": wf1,
            "wf2": wf2,
            "colb": cb,
            "natb": natb,
            "rowb": np.concatenate(
                [L[i]["bo"] for i in range(NL)] + [L[i]["bf2"] for i in range(NL)]
            ).reshape(1, -1).astype(_bf),
            "w0": w0.reshape(1, HID).astype(_bf),
            "msum": msum[lo:hi].reshape(1, VH).astype(_bf),
        }
        in_maps.append(m)
    return shared, in_maps


# ---------------------------------------------------------------------------
# Device program
# ---------------------------------------------------------------------------

_PROG_CACHE = {}


def _build_program(has_gnb, colb_idx, ncolb):
    import contextlib

    import concourse.tile as tile
    from concourse import mybir, bacc

    bf16 = mybir.dt.bfloat16
    f32 = mybir.dt.float32
    AF = mybir.ActivationFunctionType
    OP = mybir.AluOpType

    nc = bacc.Bacc("TRN2", target_bir_lowering=False, debug=False, num_devices=NCORES)

    def inp(name, shape, dt_):
        return nc.declare_dram_parameter(name, shape, dt_, isOutput=False)

    h0_d = inp("h0", [V, F0], bf16)
    mt_d = inp("mt", [V, VH], bf16)
    xskip_d = inp("xskip", [VH, HID], f32)
    cond_d = inp("cond", [NL, HID, VH], f32)
    wg0_d = inp("wg0", [F0, HID], bf16)
    wg12_d = inp("wg12", [2, HID, HID], bf16)
    wq_d = inp("wq", [NL, HID, HID], bf16)
    wk_d = inp("wk", [NL, HID, HID], bf16)
    wv_d = inp("wv", [NL, HID, HID], bf16)
    wo_d = inp("wo", [NL, HID, HID], bf16)
    wl_d = inp("wl", [NL, HID, HID], bf16)
    wf1_d = inp("wf1", [NL, HID, FF], bf16)
    wf2_d = inp("wf2", [NL, FF, HID], bf16)
    colb_d = inp("colb", [ncolb, 128], f32)
    natb_d = inp("natb", [2 * NL, 128, HID], f32)
    rowb_d = inp("rowb", [1, 12 * 128], bf16)
    w0_d = inp("w0", [1, HID], bf16)
    msum_d = inp("msum", [1, VH], bf16)
    out_d = nc.declare_dram_parameter("out", [VH, HID], f32, isOutput=True)

    kv_in = [nc.dram_tensor(f"kv_in{l}", [KVFLAT], bf16) for l in range(NL)]
    kv_out = [nc.dram_tensor(f"kv_out{l}", [2, KVFLAT], bf16) for l in range(NL)]
    hg_in = [None] + [
        [nc.dram_tensor(f"hg_in{l}_{q}", [512, HID], bf16) for q in range(2)]
        for l in (1, 2)
    ]
    hg_out = [None] + [
        [nc.dram_tensor(f"hg_out{l}_{q}", [2, 512, HID], bf16) for q in range(2)]
        for l in (1, 2)
    ]

    PAIRS = [[0, 1], [2, 3], [4, 5], [6, 7]]
    SCALE = float(1.0 / np.sqrt(DH))

    with tile.TileContext(nc) as tc, contextlib.ExitStack() as ctx:
        cp = ctx.enter_context(tc.tile_pool(name="const", bufs=1))
        wp = ctx.enter_context(tc.tile_pool(name="work", bufs=1))
        sp = ctx.enter_context(tc.tile_pool(name="small", bufs=1))
        rp = ctx.enter_context(tc.tile_pool(name="rows", bufs=2))
        esp = ctx.enter_context(tc.tile_pool(name="esp", bufs=4))
        pm = ctx.enter_context(tc.tile_pool(name="pm", bufs=2, space="PSUM"))
        psc = ctx.enter_context(tc.tile_pool(name="psc", bufs=2, space="PSUM"))
        pox = ctx.enter_context(tc.tile_pool(name="pox", bufs=1, space="PSUM"))

        # ---------------- persistent loads ----------------
        mt_sb = cp.tile([128, ST, VH], bf16)
        nc.sync.dma_start(mt_sb[:], mt_d.rearrange("(t p) d -> p t d", p=128))
        colb_sb = cp.tile([128, ncolb], f32)
        nc.sync.dma_start(colb_sb[:], colb_d.rearrange("n p -> p n"))
        natb_sb = cp.tile([128, 2 * NL, HID], f32)
        nc.sync.dma_start(natb_sb[:], natb_d.rearrange("n p f -> p n f"))

        wg0_sb = cp.tile([128, 3, HID], bf16)
        nc.vector.memset(wg0_sb[:], 0.0)
        nc.sync.dma_start(
            wg0_sb[:, 0:2, :], wg0_d[0:256].rearrange("(t p) f -> p t f", p=128)
        )
        nc.sync.dma_start(wg0_sb[:64, 2, :], wg0_d[256:320])

        rowb_sb = cp.tile([1, 12 * 128], bf16)
        nc.sync.dma_start(rowb_sb[:], rowb_d[:])
        ones_row = cp.tile([1, 512], bf16)
        nc.vector.memset(ones_row[:], 1.0)

        def rowb(name, j):
            l_ = int(name[-1])
            base = 0 if name.startswith("bo") else 6
            r = base + 2 * l_ + j
            return rowb_sb[0:1, r * 128 : (r + 1) * 128]

        w0_sb = cp.tile([1, HID], bf16)
        nc.sync.dma_start(w0_sb[:], w0_d[:])
        msum_sb = cp.tile([1, VH], bf16)
        nc.sync.dma_start(msum_sb[:], msum_d[:])

        ones_col = cp.tile([128, 1], bf16)
        nc.vector.memset(ones_col[:], 1.0)

        kTf_sb = cp.tile([128, 2, V], bf16)                 # gathered K^T
        vext_sb = cp.tile([128, ST, HEADS, DH + 1], bf16)   # gathered V nat + ones col
        nc.vector.memset(vext_sb[:], 1.0)

        def col(name, j):
            return colb_sb[:, colb_idx[name] + j : colb_idx[name] + j + 1]

        # h0 shares the full-V slot used by the gathered h of later layers
        hfull = cp.tile([128, ST, F0], bf16, tag="hfullg")
        nc.sync.dma_start(hfull[:], h0_d.rearrange("(t p) f -> p t f", p=128))
        hf_tiles = 3

        for l in range(NL):
            final = l == NL - 1

            # ---- per-layer weights ----
            wg_l = wg0_sb
            if l > 0:
                wg_l = cp.tile([128, 3, HID], bf16, tag="wg_l")
                nc.sync.dma_start(
                    wg_l[:, 0:2, :],
                    wg12_d[l - 1].rearrange("(t p) f -> p t f", p=128),
                )

            def loadw(d, kt, fo, tag):
                sb = cp.tile([128, kt, fo], bf16, tag=tag)
                nc.sync.dma_start(sb[:], d[l].rearrange("(t p) f -> p t f", p=128))
                return sb

            wq_sb = loadw(wq_d, 2, HID, "wq")
            wk_sb = loadw(wk_d, 2, HID, "wk")
            wv_sb = loadw(wv_d, 2, HID, "wv")
            wo_sb = loadw(wo_d, 2, HID, "wo")
            wl_sb = loadw(wl_d, 2, HID, "wl")
            wf1_sb = loadw(wf1_d, 2, FF, "wf1")
            wf2_sb = loadw(wf2_d, 8, HID, "wf2")
            cond_sb = cp.tile([128, 2, VH], f32, tag="cond_l")
            nc.sync.dma_start(
                cond_sb[:], cond_d[l].rearrange("(u p) v -> p u v", p=128)
            )

            # ---- gnorm (layer 1): scalar affine applied in place ----
            if l == 1:
                hsq = wp.tile([128, ST, HID], bf16, tag="hres")
                nc.vector.tensor_tensor(hsq[:], hfull[:, :, 0:HID], hfull[:, :, 0:HID], OP.mult)
                s_ps = pm.tile([128, 512], f32, tag="mm")
                q_ps = pm.tile([128, 512], f32, tag="mm")
                for st in range(ST):
                    nc.tensor.matmul(
                        s_ps[0:1, 0:HID], ones_col[:], hfull[:, st, 0:HID],
                        start=(st == 0), stop=(st == ST - 1),
                    )
                for st in range(ST):
                    nc.tensor.matmul(
                        q_ps[0:1, 0:HID], ones_col[:], hsq[:, st, :],
                        start=(st == 0), stop=(st == ST - 1),
                    )
                gn = rp.tile([1, 8], f32, tag="gn")
                nc.vector.reduce_sum(gn[:, 0:1], s_ps[0:1, 0:HID], mybir.AxisListType.X)
                nc.vector.reduce_sum(gn[:, 1:2], q_ps[0:1, 0:HID], mybir.AxisListType.X)
                VC = float(V * HID)
                nc.vector.tensor_scalar_mul(gn[:, 0:1], gn[:, 0:1], 1.0 / VC)  # mu
                nc.vector.tensor_scalar_mul(gn[:, 1:2], gn[:, 1:2], 1.0 / VC)  # E[x^2]
                nc.vector.tensor_tensor(gn[:, 2:3], gn[:, 0:1], gn[:, 0:1], OP.mult)
                nc.vector.tensor_tensor(gn[:, 2:3], gn[:, 1:2], gn[:, 2:3], OP.subtract)
                nc.vector.tensor_scalar_add(gn[:, 2:3], gn[:, 2:3], 1e-5)  # var+eps
                nc.scalar.activation(gn[:, 3:4], gn[:, 2:3], AF.Ln)
                nc.scalar.activation(gn[:, 4:5], gn[:, 3:4], AF.Exp, scale=-0.5)
                mu_c = sp.tile([128, 1], f32, tag="gmu")
                rs_c = sp.tile([128, 1], f32, tag="grs")
                nc.gpsimd.partition_broadcast(mu_c[:], gn[:, 0:1])
                nc.gpsimd.partition_broadcast(rs_c[:], gn[:, 4:5])
                nc.vector.tensor_scalar(
                    out=hfull[:, :, 0:HID], in0=hfull[:, :, 0:HID],
                    scalar1=mu_c[:], scalar2=rs_c[:],
                    op0=OP.subtract, op1=OP.mult,
                )

            # ---- gcn: zT = (M @ h)^T, then yT = (z @ Wg)^T ----
            zT = wp.tile([128, 3, VH], bf16, tag="zT")
            for ft in range(hf_tiles):
                fw = 64 if (l == 0 and ft == 2) else 128
                for qc in range(QC):
                    z_ps = pm.tile([128, 512], f32, tag="mm")
                    for st in range(ST):
                        nc.tensor.matmul(
                            z_ps[0:fw, :],
                            hfull[:, st, ft * 128 : ft * 128 + fw],
                            mt_sb[:, st, qc * 512 : (qc + 1) * 512],
                            start=(st == 0), stop=(st == ST - 1),
                        )
                    nc.scalar.activation(
                        zT[0:fw, ft, qc * 512 : (qc + 1) * 512], z_ps[0:fw, :],
                        AF.Copy,
                    )

            gnb_term = has_gnb and l == 1
            hres = wp.tile([128, 2, VH], f32, tag="hres")
            hbf = wp.tile([128, 2, VH], bf16, tag="hbf")
            for u in range(2):
                for qc in range(QC):
                    y_ps = pm.tile([128, 512], f32, tag="mm")
                    for ft in range(hf_tiles):
                        fw = 64 if (l == 0 and ft == 2) else 128
                        nc.tensor.matmul(
                            y_ps[:],
                            wg_l[0:fw, ft, u * 128 : (u + 1) * 128],
                            zT[0:fw, ft, qc * 512 : (qc + 1) * 512],
                            start=(ft == 0),
                            stop=(ft == hf_tiles - 1 and not gnb_term),
                        )
                    if gnb_term:
                        nc.tensor.matmul(
                            y_ps[:],
                            w0_sb[:, u * 128 : (u + 1) * 128],
                            msum_sb[:, qc * 512 : (qc + 1) * 512],
                            start=False, stop=True,
                        )
                    qs = slice(qc * 512, (qc + 1) * 512)
                    if not final:
                        nc.vector.tensor_scalar(
                            out=hres[:, u, qs], in0=y_ps[:], scalar1=col(f"bg{l}", u),
                            scalar2=0.0, op0=OP.add, op1=OP.max,
                        )
                    else:
                        nc.vector.tensor_scalar(
                            out=hres[:, u, qs], in0=y_ps[:], scalar1=col(f"bg{l}", u),
                            scalar2=col("gm0", u), op0=OP.add, op1=OP.mult,
                        )
                        nc.vector.tensor_scalar(
                            out=hres[:, u, qs], in0=hres[:, u, qs],
                            scalar1=col("bt0", u), scalar2=0.0,
                            op0=OP.add, op1=OP.max,
                        )
                    nc.vector.tensor_tensor(
                        hres[:, u, qs], hres[:, u, qs], cond_sb[:, u, qs], OP.add
                    )
                    nc.vector.tensor_copy(hbf[:, u, qs], hres[:, u, qs])

            # ---- k, v projections + pair allgather ----
            kT_own = wp.tile([128, 2, VH], bf16, tag="kT_own")
            for u in range(2):
                for qc in range(QC):
                    k_ps = pm.tile([128, 512], f32, tag="mm")
                    for kt in range(2):
                        nc.tensor.matmul(
                            k_ps[:],
                            wk_sb[:, kt, u * 128 : (u + 1) * 128],
                            hbf[:, kt, qc * 512 : (qc + 1) * 512],
                            start=(kt == 0), stop=(kt == 1),
                        )
                    nc.vector.tensor_scalar(
                        out=kT_own[:, u, qc * 512 : (qc + 1) * 512], in0=k_ps[:],
                        scalar1=col(f"bk{l}", u), scalar2=None, op0=OP.add,
                    )
            vnat = wp.tile([128, VS, HID], bf16, tag="vnat")
            for vs in range(VS):
                v_ps = pm.tile([128, 512], f32, tag="mm")
                for kt in range(2):
                    nc.tensor.matmul(
                        v_ps[:, 0:HID],
                        hbf[:, kt, vs * 128 : (vs + 1) * 128],
                        wv_sb[:, kt, :],
                        start=(kt == 0), stop=(kt == 1),
                    )
                nc.vector.tensor_tensor(
                    vnat[:, vs, :], v_ps[:, 0:HID], natb_sb[:, l, :], OP.add
                )
            nc.sync.dma_start(
                kv_in[l][0 : HID * VH].rearrange("(p u v) -> p u v", p=128, u=2),
                kT_own[:],
            )
            nc.sync.dma_start(
                kv_in[l][HID * VH :].rearrange("(t p) f -> p t f", p=128)
                if False else
                kv_in[l][HID * VH :].rearrange("(t p f) -> p t f", p=128, f=HID),
                vnat[:],
            )
            nc.gpsimd.collective_compute(
                "AllGather", mybir.AluOpType.bypass,
                replica_groups=PAIRS,
                ins=[kv_in[l][:].opt()], outs=[kv_out[l][:].opt()],
            )
            for g in range(2):
                nc.gpsimd.dma_start(
                    kTf_sb[:, :, g * VH : (g + 1) * VH],
                    kv_out[l][g, 0 : HID * VH].rearrange(
                        "(p u v) -> p u v", p=128, u=2
                    ),
                )
                for st8 in range(8):
                    nc.gpsimd.dma_start(
                        vext_sb[:, 8 * g + st8, :, 0:DH],
                        kv_out[l][
                            g,
                            HID * VH + st8 * 128 * HID : HID * VH + (st8 + 1) * 128 * HID,
                        ].rearrange("(p h d) -> p h d", p=128, h=HEADS),
                    )

            # ---- q projection ----
            qT = wp.tile([128, 2, VH], bf16, tag="qT")
            for u in range(2):
                for qc in range(QC):
                    q_ps = pm.tile([128, 512], f32, tag="mm")
                    for kt in range(2):
                        nc.tensor.matmul(
                            q_ps[:],
                            wq_sb[:, kt, u * 128 : (u + 1) * 128],
                            hbf[:, kt, qc * 512 : (qc + 1) * 512],
                            start=(kt == 0), stop=(kt == 1),
                        )
                    nc.vector.tensor_scalar(
                        out=qT[:, u, qc * 512 : (qc + 1) * 512], in0=q_ps[:],
                        scalar1=col(f"bq{l}", u), scalar2=None, op0=OP.add,
                    )

            # ---- attention core ----
            oT = wp.tile([128, 2, VH], bf16, tag="oT")
            for qc in range(QC):
                qs = slice(qc * 512, (qc + 1) * 512)
                for hg in range(4):          # head pairs
                    u = hg // 2
                    hh0 = (hg % 2) * 2
                    o_ps = pox.tile([DH + 1, 2, 512], f32, tag="oext")
                    for st in range(ST):
                        s_ps = psc.tile([128, 2, 512], f32, tag="sc")
                        for j in range(2):
                            hh = hh0 + j
                            nc.tensor.matmul(
                                s_ps[:, j, :],
                                kTf_sb[32 * hh : 32 * hh + 32, u,
                                       st * 128 : (st + 1) * 128],
                                qT[32 * hh : 32 * hh + 32, u, qs],
                                start=True, stop=True,
                                tile_position=(32 * hh, 0),
                            )
                        es = esp.tile([128, 2, 512], bf16, tag="es")
                        nc.scalar.activation(es[:], s_ps[:], AF.Exp, scale=SCALE)
                        for j in range(2):
                            h_abs = u * 4 + hh0 + j
                            nc.tensor.matmul(
                                o_ps[:, j, :],
                                vext_sb[:, st, h_abs, :],
                                es[:, j, :],
                                start=(st == 0), stop=(st == ST - 1),
                            )
                    srow = rp.tile([1, 2, 512], f32, tag="avs")
                    nc.vector.tensor_copy(srow[:], o_ps[DH : DH + 1, :, :])
                    sbc = rp.tile([DH, 2, 512], f32, tag="avb")
                    nc.gpsimd.partition_broadcast(sbc[:], srow[:])
                    nc.vector.reciprocal_approx_fast(out=sbc[:], in_=sbc[:])
                    for j in range(2):
                        hh = hh0 + j
                        nc.vector.tensor_tensor(
                            oT[32 * hh : 32 * hh + 32, u, qs],
                            o_ps[0:DH, j, :], sbc[:, j, :], OP.mult,
                        )
            # ---- h = h + o @ Wo + bo ----
            for u in range(2):
                for qc in range(QC):
                    qs = slice(qc * 512, (qc + 1) * 512)
                    w_ps = pm.tile([128, 512], f32, tag="mm")
                    for kt in range(2):
                        nc.tensor.matmul(
                            w_ps[:],
                            wo_sb[:, kt, u * 128 : (u + 1) * 128],
                            oT[:, kt, qs],
                            start=(kt == 0), stop=False,
                        )
                    nc.tensor.matmul(
                        w_ps[:], rowb(f"bo{l}", u), ones_row[:],
                        start=False, stop=True,
                    )
                    nc.vector.tensor_tensor(
                        hres[:, u, qs], hres[:, u, qs], w_ps[:], OP.add
                    )
                    nc.vector.tensor_copy(hbf[:, u, qs], hres[:, u, qs])

            # ---- ff ----
            for qc in range(QC):
                qs = slice(qc * 512, (qc + 1) * 512)
                f1 = wp.tile([128, 8, 512], bf16, tag="f1")
                for m in range(8):
                    f_ps = pm.tile([128, 512], f32, tag="mm")
                    for kt in range(2):
                        nc.tensor.matmul(
                            f_ps[:],
                            wf1_sb[:, kt, m * 128 : (m + 1) * 128],
                            hbf[:, kt, qs],
                            start=(kt == 0), stop=(kt == 1),
                        )
                    nc.scalar.activation(
                        f1[:, m, :], f_ps[:], AF.Relu, bias=col(f"bf1_{l}", m),
                    )
                for u in range(2):
                    g_ps = pm.tile([128, 512], f32, tag="mm")
                    for kt in range(8):
                        nc.tensor.matmul(
                            g_ps[:],
                            wf2_sb[:, kt, u * 128 : (u + 1) * 128],
                            f1[:, kt, :],
                            start=(kt == 0), stop=False,
                        )
                    nc.tensor.matmul(
                        g_ps[:], rowb(f"bf2_{l}", u), ones_row[:],
                        start=False, stop=True,
                    )
                    nc.vector.tensor_tensor(
                        hres[:, u, qs], hres[:, u, qs], g_ps[:], OP.add
                    )
                    nc.vector.tensor_copy(hbf[:, u, qs], hres[:, u, qs])

            # ---- layernorm over channels (whole layer) ----
            hsq2v = wp.tile([128, 2, VH], bf16, tag="hsq")
            nc.vector.tensor_tensor(hsq2v[:], hbf[:], hbf[:], OP.mult)
            s1 = sp.tile([1, VH], f32, tag="lns1")
            s2 = sp.tile([1, VH], f32, tag="lns2")
            for qc in range(QC):
                r_ps = pm.tile([128, 512], f32, tag="mm")
                for u in range(2):
                    nc.tensor.matmul(
                        r_ps[0:1, :], ones_col[:],
                        hbf[:, u, qc * 512 : (qc + 1) * 512],
                        start=(u == 0), stop=(u == 1),
                    )
                nc.vector.tensor_scalar_mul(
                    s1[:, qc * 512 : (qc + 1) * 512], r_ps[0:1, :], 1.0 / HID
                )
                r2_ps = pm.tile([128, 512], f32, tag="mm")
                for u in range(2):
                    nc.tensor.matmul(
                        r2_ps[0:1, :], ones_col[:],
                        hsq2v[:, u, qc * 512 : (qc + 1) * 512],
                        start=(u == 0), stop=(u == 1),
                    )
                nc.vector.tensor_scalar_mul(
                    s2[:, qc * 512 : (qc + 1) * 512], r2_ps[0:1, :], 1.0 / HID
                )
            var = sp.tile([1, VH], f32, tag="lnvar")
            nc.vector.tensor_tensor(var[:], s1[:], s1[:], OP.mult)
            nc.vector.tensor_tensor(var[:], s2[:], var[:], OP.subtract)
            nc.vector.tensor_scalar_add(var[:], var[:], 1e-5)
            lnv = sp.tile([1, VH], f32, tag="lnlog")
            nc.scalar.activation(lnv[:], var[:], AF.Ln)
            rstd = var
            nc.scalar.activation(rstd[:], lnv[:], AF.Exp, scale=-0.5)
            nb = s2
            nc.vector.tensor_tensor(nb[:], s1[:], rstd[:], OP.mult)
            nc.vector.tensor_scalar_mul(nb[:], nb[:], -1.0)
            a_bc = sp.tile([128, VH], f32, tag="lnabc")
            b_bc = sp.tile([128, VH], f32, tag="lnbbc")
            nc.gpsimd.partition_broadcast(a_bc[:], rstd[:])
            nc.gpsimd.partition_broadcast(b_bc[:], nb[:])
            hln = wp.tile([128, 2, VH], bf16, tag="hln")
            for u in range(2):
                nc.vector.tensor_tensor(
                    hres[:, u, :], hres[:, u, :], a_bc[:], OP.mult
                )
                nc.vector.tensor_tensor(
                    hres[:, u, :], hres[:, u, :], b_bc[:], OP.add
                )
                nc.vector.tensor_scalar(
                    out=hln[:, u, :], in0=hres[:, u, :], scalar1=col(f"lng{l}", u),
                    scalar2=col(f"lnb{l}", u), op0=OP.mult, op1=OP.add,
                )

            # ---- Wl linear (natural out) ----
            if not final:
                hnat_bf = wp.tile([128, VS, HID], bf16, tag="hnatb")
                for qh in range(2):
                    for vi in range(4):
                        vs = qh * 4 + vi
                        n_ps = pm.tile([128, 512], f32, tag="mm")
                        for kt in range(2):
                            nc.tensor.matmul(
                                n_ps[:, 0:HID],
                                hln[:, kt, vs * 128 : (vs + 1) * 128],
                                wl_sb[:, kt, :],
                                start=(kt == 0), stop=(kt == 1),
                            )
                        tmp = sp.tile([128, HID], f32, tag="wltmp")
                        nc.vector.tensor_tensor(
                            tmp[:], n_ps[:, 0:HID], natb_sb[:, NL + l, :], OP.add
                        )
                        nc.vector.tensor_scalar(
                            out=hnat_bf[:, vs, :], in0=tmp[:], scalar1=0.0,
                            scalar2=None, op0=OP.max,
                        )
                    nc.sync.dma_start(
                        hg_in[l + 1][qh][:].rearrange("(t p) f -> p t f", p=128),
                        hnat_bf[:, qh * 4 : qh * 4 + 4, :],
                    )
            else:
                for vs in range(VS):
                    n_ps = pm.tile([128, 512], f32, tag="mm")
                    for kt in range(2):
                        nc.tensor.matmul(
                            n_ps[:, 0:HID],
                            hln[:, kt, vs * 128 : (vs + 1) * 128],
                            wl_sb[:, kt, :],
                            start=(kt == 0), stop=(kt == 1),
                        )
                    xs = sp.tile([128, HID], f32, tag="xstmp")
                    nc.sync.dma_start(xs[:], xskip_d[vs * 128 : (vs + 1) * 128, :])
                    tmp = sp.tile([128, HID], f32, tag="wltmp")
                    nc.vector.tensor_tensor(
                        tmp[:], n_ps[:, 0:HID], natb_sb[:, NL + l, :], OP.add
                    )
                    out2 = sp.tile([128, HID], f32, tag="outtmp")
                    nc.vector.tensor_tensor(out2[:], tmp[:], xs[:], OP.add)
                    nc.sync.dma_start(
                        out_d[vs * 128 : (vs + 1) * 128, :], out2[:]
                    )

            if not final:
                hfull = cp.tile([128, ST, F0], bf16, tag="hfullg")
                for qh in range(2):
                    nc.gpsimd.collective_compute(
                        "AllGather", mybir.AluOpType.bypass,
                        replica_groups=PAIRS,
                        ins=[hg_in[l + 1][qh][:].opt()],
                        outs=[hg_out[l + 1][qh][:].opt()],
                    )
                    for g in range(2):
                        nc.gpsimd.dma_start(
                            hfull[:, 8 * g + 4 * qh : 8 * g + 4 * qh + 4, 0:HID],
                            hg_out[l + 1][qh][g].rearrange(
                                "(t p) f -> p t f", p=128
                            ),
                        )
                hf_tiles = 2

    nc.finalize()
    return nc


LAST_EXEC_NS = None


def kernel(x, cond_x, edge_index, t, params):
    import os
    from concourse.bass_utils import run_bass_kernel_spmd

    shared, in_maps = _host_prep(x, cond_x, edge_index, t, params)

    key = (shared["has_gnb"],)
    if key not in _PROG_CACHE:
        _PROG_CACHE[key] = _build_program(
            shared["has_gnb"], shared["colb_idx"], shared["ncolb"]
        )
    nc = _PROG_CACHE[key]

    trace = os.environ.get("ATTGNN_TRACE", "0") == "1"
    r = run_bass_kernel_spmd(nc, in_maps, list(range(NCORES)), trace=trace)
    global LAST_EXEC_NS
    LAST_EXEC_NS = r.exec_time_ns
    out = np.zeros((B, V, HID), np.float32)
    for core in range(NCORES):
        b, rr = core // 2, core % 2
        out[b, rr * VH : (rr + 1) * VH] = r.results[core]["out"]
    return out
